# revision 1
# baseline (speedup 1.0000x reference)
"""DAWNBlock Trainium2 kernel (8 NeuronCores, SPMD, single NEFF launch).

Sharding: tokens split over cores as (batch b = c//2, seq-half hf = c%2),
512 tokens per core. Attention is sharded by (batch, head-group): after a
pair AllGather of Q^T/K^T/V each core runs causal attention for 8 heads over
the full 1024-token sequence of its batch; a second pair AllGather exchanges
attn^T so each core projects (W_O) only its own 512 tokens. The knowledge
stage is token-parallel: knowledge_K^T is streamed through SBUF in bf16,
scores are computed in 32 chunks of 1024, and top-8 selection uses the
hardware max8 instruction over packed floats (bf16 score in the high 16
bits, global index in the low 15 mantissa bits), followed by per-partition
indirect-DMA row gathers from knowledge_V.

Core-parity-dependent data movement (which half of the AllGather output
belongs to this core) is handled with register-backed dynamic DMA slices
(bass.ds) driven by a tiny per-core offsets input, so all 8 cores share one
instruction stream.
"""
import functools
import numpy as np
import ml_dtypes

import concourse.bass as bass
import concourse.bacc as bacc
import concourse.mybir as mybir
import concourse.tile as tile
from concourse.bass_utils import run_bass_kernel_spmd

F32 = mybir.dt.float32
BF16 = mybir.dt.bfloat16
U32 = mybir.dt.uint32
U16 = mybir.dt.uint16
AF = mybir.ActivationFunctionType
OP = mybir.AluOpType
AX = mybir.AxisListType

N_CORES = 8
P = 128
D = 1024
R = 128
NCMP = 16
NK = 32768
KK = 8
S = 1024
B = 4
TOK = 512
NT = TOK // P          # 4 token tiles per core
EPS = 1e-5
NEG = -1.0e30
KC = 1024              # knowledge-score chunk width
NKC = NK // KC         # 32 chunks
SCALE_R = float(1.0 / np.sqrt(R))


def _ln(nc, sb, x_ap, out_ap, eps_tile):
    """LayerNorm (gamma=1, beta=0): x_ap [128, D] f32 -> out_ap (bf16)."""
    stats = sb.tile([P, 2, 6], F32, tag="ln_stats")
    for g in range(2):
        nc.vector.bn_stats(out=stats[:, g, :], in_=x_ap[:, g * 512:(g + 1) * 512])
    mv = sb.tile([P, 2], F32, tag="ln_mv")
    nc.vector.bn_aggr(out=mv[:], in_=stats[:])
    rstd = sb.tile([P, 1], F32, tag="ln_rstd")
    nc.scalar.activation(out=rstd[:], in_=mv[:, 1:2], func=AF.Sqrt,
                         bias=eps_tile[:], scale=1.0)
    nc.vector.reciprocal(out=rstd[:], in_=rstd[:])
    nc.vector.tensor_scalar(out=out_ap, in0=x_ap, scalar1=mv[:, 0:1],
                            scalar2=rstd[:], op0=OP.subtract, op1=OP.mult)


def _softmax16(nc, sb, logits_ap, w_ap):
    """softmax over 16 router logits (PSUM f32 in) -> w_ap [128,16] f32."""
    mx = sb.tile([P, 1], F32, tag="rs_mx")
    nc.vector.tensor_reduce(out=mx[:], in_=logits_ap, axis=AX.X, op=OP.max)
    nmx = sb.tile([P, 1], F32, tag="rs_nmx")
    nc.vector.tensor_scalar_mul(out=nmx[:], in0=mx[:], scalar1=-1.0)
    ssum = sb.tile([P, 1], F32, tag="rs_sum")
    nc.scalar.activation(out=w_ap, in_=logits_ap, func=AF.Exp,
                         bias=nmx[:], scale=1.0, accum_out=ssum[:])
    nc.vector.reciprocal(out=ssum[:], in_=ssum[:])
    nc.vector.tensor_scalar_mul(out=w_ap, in0=w_ap, scalar1=ssum[:])


def _combine(nc, sb, p1_halves, w_ap, out_ap):
    """out[t,:] = sum_n w[t,n] * P1[t, n*128:(n+1)*128] (P1 in 2 PSUM halves)."""
    acc = sb.tile([P, R], F32, tag="cmb_acc")
    for n in range(NCMP):
        src = p1_halves[n // 8][:, (n % 8) * R:(n % 8 + 1) * R]
        if n == 0:
            nc.vector.tensor_scalar(out=acc[:], in0=src, scalar1=w_ap[:, 0:1],
                                    scalar2=None, op0=OP.mult)
        else:
            nc.vector.scalar_tensor_tensor(out=acc[:], in0=src,
                                           scalar=w_ap[:, n:n + 1], in1=acc[:],
                                           op0=OP.mult, op1=OP.add)
    nc.vector.tensor_copy(out=out_ap, in_=acc[:])


def build_program():
    nc = bacc.Bacc(None, num_devices=N_CORES)

    x_in = nc.dram_tensor("x_shard", [TOK, D], F32, kind="ExternalInput")
    tri_in = nc.dram_tensor("tri", [P, P], F32, kind="ExternalInput")
    neur_in = nc.dram_tensor("neurons", [D, NCMP * R], BF16, kind="ExternalInput")
    rQKV_in = nc.dram_tensor("routersQKV", [D, 48], BF16, kind="ExternalInput")
    rM_in = nc.dram_tensor("routerM", [D, NCMP], BF16, kind="ExternalInput")
    wq_in = nc.dram_tensor("W_Q", [R, D], BF16, kind="ExternalInput")
    wk_in = nc.dram_tensor("W_K", [R, D], BF16, kind="ExternalInput")
    wv_in = nc.dram_tensor("W_V", [R, D], BF16, kind="ExternalInput")
    wo_in = nc.dram_tensor("W_O", [D, D], BF16, kind="ExternalInput")
    kKT_in = nc.dram_tensor("kKT", [R, NK], BF16, kind="ExternalInput")
    kV_in = nc.dram_tensor("kV", [NK, D], BF16, kind="ExternalInput")
    id_in = nc.dram_tensor("ident", [P, P], BF16, kind="ExternalInput")
    offs_in = nc.dram_tensor("offs", [1, 2], U32, kind="ExternalInput")
    out_t = nc.dram_tensor("out_shard", [TOK, D], F32, kind="ExternalOutput")
    dbg_x2 = nc.dram_tensor("dbg_x2", [TOK, D], F32, kind="ExternalOutput")
    dbg_h = nc.dram_tensor("dbg_h", [P, D], F32, kind="ExternalOutput")
    dbg_hq = nc.dram_tensor("dbg_hq", [P, R], F32, kind="ExternalOutput")
    dbg_idx = nc.dram_tensor("dbg_idx", [P, 8], U32, kind="ExternalOutput")
    dbg_vals = nc.dram_tensor("dbg_vals", [P, 8], F32, kind="ExternalOutput")
    dbg_qm = nc.dram_tensor("dbg_qm", [P, R], F32, kind="ExternalOutput")
    dbg_pk = nc.dram_tensor("dbg_pk", [P, KC], U32, kind="ExternalOutput")

    with tile.TileContext(nc) as tc:
        with (
            tc.tile_pool(name="persist", bufs=1) as pp,
            tc.tile_pool(name="weights", bufs=1) as wp,
            tc.tile_pool(name="work", bufs=2) as sb,
            tc.tile_pool(name="gath", bufs=3) as gp,
            tc.tile_pool(name="ps_big", bufs=2, space="PSUM") as psb,
            tc.tile_pool(name="ps_tp", bufs=2, space="PSUM") as pst,
            tc.tile_pool(name="ps_sm", bufs=2, space="PSUM") as psa,
            tc.tile_pool(name="dram", bufs=1, space="DRAM") as dram,
        ):
            # ---- parity offsets -> gpsimd registers for dynamic DMA slices ----
            r2048 = nc.gpsimd.alloc_register("off2048")
            nc.gpsimd.reg_load(r2048, offs_in[0:1, 0:1])
            off2048 = nc.gpsimd.snap(r2048, donate=True, min_val=0, max_val=2048)
            r512 = nc.gpsimd.alloc_register("off512")
            nc.gpsimd.reg_load(r512, offs_in[0:1, 1:2])
            off512 = nc.gpsimd.snap(r512, donate=True, min_val=0, max_val=512)

            # ---- resident weights ----
            neur = wp.tile([P, 8, NCMP * R], BF16)
            nc.sync.dma_start(out=neur[:], in_=neur_in[:].rearrange("(c p) n -> p c n", p=P))
            rQKV = wp.tile([P, 8, 48], BF16)
            nc.sync.dma_start(out=rQKV[:], in_=rQKV_in[:].rearrange("(c p) n -> p c n", p=P))
            rM = wp.tile([P, 8, NCMP], BF16)
            nc.sync.dma_start(out=rM[:], in_=rM_in[:].rearrange("(c p) n -> p c n", p=P))
            wq = wp.tile([P, D], BF16)
            nc.sync.dma_start(out=wq[:], in_=wq_in[:])
            wk = wp.tile([P, D], BF16)
            nc.sync.dma_start(out=wk[:], in_=wk_in[:])
            wv = wp.tile([P, D], BF16)
            nc.sync.dma_start(out=wv[:], in_=wv_in[:])
            wo = wp.tile([P, 8, D], BF16)
            nc.sync.dma_start(out=wo[:], in_=wo_in[:].rearrange("(c p) n -> p c n", p=P))
            ident = wp.tile([P, P], BF16)
            nc.sync.dma_start(out=ident[:], in_=id_in[:])
            tri = wp.tile([P, P], F32)
            nc.sync.dma_start(out=tri[:], in_=tri_in[:])
            eps_t = wp.tile([P, 1], F32)
            nc.vector.memset(eps_t[:], EPS)
            iota_t = wp.tile([P, KC], U16)
            nc.gpsimd.iota(out=iota_t[:], pattern=[[64, KC]], base=0,
                           channel_multiplier=0)

            # ---- persistent activations ----
            x_all = pp.tile([P, NT, D], F32)
            hT = pp.tile([P, 8, TOK], BF16, tag="hT")
            hQT = pp.tile([P, TOK], BF16, tag="hQT")
            hKT = pp.tile([P, TOK], BF16, tag="hKT")
            hVT = pp.tile([P, TOK], BF16, tag="hVT")
            QT_sb = pp.tile([P, 8, TOK], BF16, tag="qt")
            KT_sb = pp.tile([P, 8, TOK], BF16, tag="kt")
            V_sb = pp.tile([P, NT, D], BF16, tag="vv")

            # packed score buffers (iota pre-written into the low u16 lanes)
            packed = [pp.tile([P, KC], U32, tag=f"pk{i}", name=f"pk{i}") for i in range(3)]
            for pk in packed:
                nc.vector.tensor_copy(out=pk.bitcast(U16)[:, 0::2], in_=iota_t[:])

            # =========== S1: LN1, shared projection, routed compress ===========
            for t in range(NT):
                ts = slice(t * P, (t + 1) * P)
                nc.sync.dma_start(out=x_all[:, t, :], in_=x_in[ts, :])
                h = sb.tile([P, D], BF16, tag="h")
                _ln(nc, sb, x_all[:, t, :], h[:], eps_t)
                if t == 0:
                    dbg_h_sb = sb.tile([P, D], F32, tag="dbgh")
                    nc.vector.tensor_copy(out=dbg_h_sb[:], in_=h[:])
                    nc.sync.dma_start(out=dbg_h[:], in_=dbg_h_sb[:])
                for ch in range(8):
                    tp = pst.tile([P, P], BF16, tag="tp")
                    nc.tensor.transpose(out=tp[:], in_=h[:, ch * P:(ch + 1) * P],
                                        identity=ident[:])
                    nc.scalar.activation(out=hT[:, ch, ts], in_=tp[:], func=AF.Copy)
                lg = psa.tile([P, 48], F32, tag="sm")
                for ch in range(8):
                    nc.tensor.matmul(out=lg[:], lhsT=hT[:, ch, ts], rhs=rQKV[:, ch, :],
                                     start=(ch == 0), stop=(ch == 7))
                wQKV = sb.tile([P, 48], F32, tag="wQKV")
                for rr in range(3):
                    _softmax16(nc, sb, lg[:, rr * 16:(rr + 1) * 16],
                               wQKV[:, rr * 16:(rr + 1) * 16])
                p1a = psb.tile([P, KC], F32, tag="big")
                p1b = psb.tile([P, KC], F32, tag="big")
                for half, pt in ((0, p1a), (1, p1b)):
                    for col in range(2):
                        c0 = half * KC + col * 512
                        for ch in range(8):
                            nc.tensor.matmul(out=pt[:, col * 512:(col + 1) * 512],
                                             lhsT=hT[:, ch, ts],
                                             rhs=neur[:, ch, c0:c0 + 512],
                                             start=(ch == 0), stop=(ch == 7))
                for rr, dst in ((0, hQT), (1, hKT), (2, hVT)):
                    hc = sb.tile([P, R], BF16, tag="hc")
                    _combine(nc, sb, (p1a, p1b), wQKV[:, rr * 16:(rr + 1) * 16], hc[:])
                    if t == 0 and rr == 0:
                        dbg_hq_sb = sb.tile([P, R], F32, tag="dbghq")
                        nc.vector.tensor_copy(out=dbg_hq_sb[:], in_=hc[:])
                        nc.sync.dma_start(out=dbg_hq[:], in_=dbg_hq_sb[:])
                    tp = pst.tile([P, P], BF16, tag="tp")
                    nc.tensor.transpose(out=tp[:], in_=hc[:], identity=ident[:])
                    nc.scalar.activation(out=dst[:, ts], in_=tp[:], func=AF.Copy)

            # =========== S2: Q^T / K^T (all 16 heads) and V ===========
            for ch in range(8):
                for w_, hsrc, dst in ((wq, hQT, QT_sb), (wk, hKT, KT_sb)):
                    pr = pst.tile([P, TOK], F32, tag="tp")
                    nc.tensor.matmul(out=pr[:], lhsT=w_[:, ch * P:(ch + 1) * P],
                                     rhs=hsrc[:], start=True, stop=True)
                    nc.scalar.activation(out=dst[:, ch, :], in_=pr[:], func=AF.Copy)
            for t in range(NT):
                pv = psb.tile([P, D], F32, tag="big")
                for col in range(2):
                    nc.tensor.matmul(out=pv[:, col * 512:(col + 1) * 512],
                                     lhsT=hVT[:, t * P:(t + 1) * P],
                                     rhs=wv[:, col * 512:(col + 1) * 512],
                                     start=True, stop=True)
                nc.scalar.activation(out=V_sb[:, t, :], in_=pv[:], func=AF.Copy)

            # =========== S3: pair AllGather of QT/KT/V ===========
            groups = [[0, 1], [2, 3], [4, 5], [6, 7]]
            xin = dram.tile([P, 12288], BF16)
            xout = dram.tile([2 * P, 12288], BF16)
            nc.gpsimd.dma_start(out=xin[:, 0:4096],
                                in_=QT_sb[:].rearrange("p c t -> p (c t)"))
            nc.gpsimd.dma_start(out=xin[:, 4096:8192],
                                in_=KT_sb[:].rearrange("p c t -> p (c t)"))
            nc.gpsimd.dma_start(out=xin[:, 8192:12288],
                                in_=V_sb[:].rearrange("p c t -> p (c t)"))
            nc.gpsimd.collective_compute("AllGather", OP.bypass,
                                         replica_groups=groups,
                                         ins=[xin.opt()], outs=[xout.opt()])
            # reuse the big persistent slots for the assembled full-seq tensors
            QT_f = pp.tile([P, 4, S], BF16, tag="qt")
            KT_f = pp.tile([P, 4, S], BF16, tag="kt")
            V_f = pp.tile([P, 8, 512], BF16, tag="vv")
            for src in range(2):
                rs = slice(src * P, (src + 1) * P)
                qsl = slice(src * TOK, (src + 1) * TOK)
                for i in range(4):
                    nc.gpsimd.dma_start(
                        out=QT_f[:, i, qsl],
                        in_=xout[rs, 0:4096][:, bass.ds(off2048 + i * TOK, TOK)])
                    nc.gpsimd.dma_start(
                        out=KT_f[:, i, qsl],
                        in_=xout[rs, 4096:8192][:, bass.ds(off2048 + i * TOK, TOK)])
                    nc.gpsimd.dma_start(
                        out=V_f[:, src * 4 + i, :],
                        in_=xout[rs, 8192:12288][:, bass.ds(off512 + i * D, 512)])

            # =========== S4: causal attention, 8 heads, full sequence ===========
            attnT = pp.tile([P, 4, S], BF16, tag="at")
            for hh in range(8):
                ch, poff = hh // 2, (hh % 2) * 64
                prow = slice(poff, poff + 64)
                for qg in range(8):
                    kr = (qg + 1) * P
                    sc = psb.tile([P, S], F32, tag="big")
                    for part in range((kr + 511) // 512):
                        k0, k1 = part * 512, min(kr, (part + 1) * 512)
                        nc.tensor.matmul(out=sc[:, k0:k1],
                                         lhsT=QT_f[prow, ch, qg * P:(qg + 1) * P],
                                         rhs=KT_f[prow, ch, k0:k1],
                                         start=True, stop=True)
                    mtmp = sb.tile([P, P], F32, tag="mtmp")
                    nc.vector.tensor_tensor(out=mtmp[:], in0=sc[:, qg * P:kr],
                                            in1=tri[:], op=OP.add)
                    Pb = sb.tile([P, S], BF16, tag="Pb")
                    s2 = sb.tile([P, 1], F32, tag="s2")
                    if qg > 0:
                        s1 = sb.tile([P, 1], F32, tag="s1")
                        nc.scalar.activation(out=Pb[:, 0:qg * P], in_=sc[:, 0:qg * P],
                                             func=AF.Exp, scale=0.125, accum_out=s1[:])
                    nc.scalar.activation(out=Pb[:, qg * P:kr], in_=mtmp[:],
                                         func=AF.Exp, scale=0.125, accum_out=s2[:])
                    den = sb.tile([P, 1], F32, tag="den")
                    if qg > 0:
                        nc.vector.tensor_tensor(out=den[:], in0=s1[:], in1=s2[:],
                                                op=OP.add)
                    else:
                        nc.vector.tensor_copy(out=den[:], in_=s2[:])
                    nc.vector.reciprocal(out=den[:], in_=den[:])
                    diag = sb.tile([P, P], BF16, tag="diag")
                    nc.vector.tensor_tensor(out=diag[:], in0=ident[:],
                                            in1=den[:].to_broadcast([P, P]),
                                            op=OP.mult)
                    at = psa.tile([64, P], F32, tag="sm")
                    for kb in range(qg + 1):
                        ptp = pst.tile([P, P], F32, tag="tp")
                        nc.tensor.matmul(out=ptp[:],
                                         lhsT=Pb[:, kb * P:(kb + 1) * P],
                                         rhs=diag[:], start=True, stop=True)
                        pts = sb.tile([P, P], BF16, tag="pts")
                        nc.scalar.activation(out=pts[:], in_=ptp[:], func=AF.Copy)
                        nc.tensor.matmul(out=at[:],
                                         lhsT=V_f[:, kb, hh * 64:(hh + 1) * 64],
                                         rhs=pts[:], start=(kb == 0), stop=(kb == qg))
                    nc.scalar.activation(out=attnT[prow, ch, qg * P:(qg + 1) * P],
                                         in_=at[:], func=AF.Copy)

            # =========== S5: exchange attn^T, W_O, residual ===========
            xin2 = dram.tile([P, 4 * S], BF16)
            xout2 = dram.tile([2 * P, 4 * S], BF16)
            nc.gpsimd.dma_start(out=xin2[:], in_=attnT[:].rearrange("p c q -> p (c q)"))
            nc.gpsimd.collective_compute("AllGather", OP.bypass,
                                         replica_groups=groups,
                                         ins=[xin2.opt()], outs=[xout2.opt()])
            aT = pp.tile([P, 8, TOK], BF16, tag="at")
            for src in range(2):
                rs = slice(src * P, (src + 1) * P)
                for i in range(4):
                    nc.gpsimd.dma_start(
                        out=aT[:, src * 4 + i, :],
                        in_=xout2[rs, :][:, bass.ds(off512 + i * S, TOK)])
            for t in range(NT):
                ts = slice(t * P, (t + 1) * P)
                po = psb.tile([P, D], F32, tag="big")
                for col in range(2):
                    for ch in range(8):
                        nc.tensor.matmul(out=po[:, col * 512:(col + 1) * 512],
                                         lhsT=aT[:, ch, ts],
                                         rhs=wo[:, ch, col * 512:(col + 1) * 512],
                                         start=(ch == 0), stop=(ch == 7))
                nc.vector.tensor_tensor(out=x_all[:, t, :], in0=po[:],
                                        in1=x_all[:, t, :], op=OP.add)
                nc.sync.dma_start(out=dbg_x2[ts, :], in_=x_all[:, t, :])

            # =========== S6: LN2 + compress M -> Qm^T (into hQT) ===========
            for t in range(NT):
                ts = slice(t * P, (t + 1) * P)
                h2 = sb.tile([P, D], BF16, tag="h")
                _ln(nc, sb, x_all[:, t, :], h2[:], eps_t)
                for ch in range(8):
                    tp = pst.tile([P, P], BF16, tag="tp")
                    nc.tensor.transpose(out=tp[:], in_=h2[:, ch * P:(ch + 1) * P],
                                        identity=ident[:])
                    nc.scalar.activation(out=hT[:, ch, ts], in_=tp[:], func=AF.Copy)
                lgm = psa.tile([P, NCMP], F32, tag="sm")
                for ch in range(8):
                    nc.tensor.matmul(out=lgm[:], lhsT=hT[:, ch, ts], rhs=rM[:, ch, :],
                                     start=(ch == 0), stop=(ch == 7))
                wM = sb.tile([P, NCMP], F32, tag="wM")
                _softmax16(nc, sb, lgm[:], wM[:])
                p1a = psb.tile([P, KC], F32, tag="big")
                p1b = psb.tile([P, KC], F32, tag="big")
                for half, pt in ((0, p1a), (1, p1b)):
                    for col in range(2):
                        c0 = half * KC + col * 512
                        for ch in range(8):
                            nc.tensor.matmul(out=pt[:, col * 512:(col + 1) * 512],
                                             lhsT=hT[:, ch, ts],
                                             rhs=neur[:, ch, c0:c0 + 512],
                                             start=(ch == 0), stop=(ch == 7))
                qm = sb.tile([P, R], BF16, tag="hc")
                _combine(nc, sb, (p1a, p1b), wM[:], qm[:])
                if t == 0:
                    dbg_qm_sb = sb.tile([P, R], F32, tag="dbgqm")
                    nc.vector.tensor_copy(out=dbg_qm_sb[:], in_=qm[:])
                    nc.sync.dma_start(out=dbg_qm[:], in_=dbg_qm_sb[:])
                tp = pst.tile([P, P], BF16, tag="tp")
                nc.tensor.transpose(out=tp[:], in_=qm[:], identity=ident[:])
                nc.scalar.activation(out=hQT[:, ts], in_=tp[:], func=AF.Copy)

            # =========== S7: knowledge scores, top-8, gather, output ===========
            cands = [pp.tile([P, NKC * 8], U32, tag=f"cand{t}", name=f"cand{t}") for t in range(NT)]
            for c in range(NKC):
                kch = gp.tile([P, KC], BF16, tag="kch")
                nc.sync.dma_start(out=kch[:], in_=kKT_in[:, c * KC:(c + 1) * KC])
                for t in range(NT):
                    ts = slice(t * P, (t + 1) * P)
                    ks = psb.tile([P, KC], F32, tag="big")
                    for col in range(2):
                        nc.tensor.matmul(out=ks[:, col * 512:(col + 1) * 512],
                                         lhsT=hQT[:, ts],
                                         rhs=kch[:, col * 512:(col + 1) * 512],
                                         start=True, stop=True)
                    pk = packed[(c * NT + t) % 3]
                    nc.scalar.activation(out=pk.bitcast(U16)[:, 1::2].bitcast(BF16),
                                         in_=ks[:], func=AF.Copy)
                    if c == 0 and t == 0:
                        nc.sync.dma_start(out=dbg_pk[:], in_=pk[:])
                    c8 = cands[t][:, c * 8:(c + 1) * 8]
                    nc.vector.max(out=c8.bitcast(F32), in_=pk.bitcast(F32)[:])
            for t in range(NT):
                ts = slice(t * P, (t + 1) * P)
                top8 = sb.tile([P, 8], F32, tag="top8")
                nc.vector.max(out=top8[:], in_=cands[t].bitcast(F32)[:])
                pos = sb.tile([P, 8], U32, tag="pos")
                nc.vector.max_index(out=pos[:], in_max=top8[:],
                                    in_values=cands[t].bitcast(F32)[:])
                loc = sb.tile([P, 8], U32, tag="loc")
                nc.vector.tensor_scalar(out=loc[:], in0=top8[:].bitcast(U32),
                                        scalar1=6, scalar2=0x3FF,
                                        op0=OP.logical_shift_right,
                                        op1=OP.bitwise_and)
                cb = sb.tile([P, 8], U32, tag="cb")
                nc.vector.tensor_scalar(out=cb[:], in0=pos[:],
                                        scalar1=3, scalar2=10,
                                        op0=OP.logical_shift_right,
                                        op1=OP.logical_shift_left)
                idx = sb.tile([P, 8], U32, tag="idx")
                nc.vector.tensor_tensor(out=idx[:], in0=cb[:], in1=loc[:],
                                        op=OP.bitwise_or)
                vals = sb.tile([P, 8], F32, tag="vals")
                nc.vector.tensor_scalar(out=vals[:].bitcast(U32),
                                        in0=top8[:].bitcast(U32),
                                        scalar1=0xFFFF0000, scalar2=None,
                                        op0=OP.bitwise_and)
                nc.vector.tensor_scalar_mul(out=vals[:], in0=vals[:], scalar1=SCALE_R)
                if t == 0:
                    nc.sync.dma_start(out=dbg_idx[:], in_=idx[:])
                    nc.sync.dma_start(out=dbg_vals[:], in_=vals[:])
                mx8 = sb.tile([P, 1], F32, tag="mx8")
                nc.vector.tensor_reduce(out=mx8[:], in_=vals[:], axis=AX.X, op=OP.max)
                nmx8 = sb.tile([P, 1], F32, tag="nmx8")
                nc.vector.tensor_scalar_mul(out=nmx8[:], in0=mx8[:], scalar1=-1.0)
                kw = sb.tile([P, 8], F32, tag="kw")
                ks8 = sb.tile([P, 1], F32, tag="ks8")
                nc.scalar.activation(out=kw[:], in_=vals[:], func=AF.Exp,
                                     bias=nmx8[:], scale=1.0, accum_out=ks8[:])
                nc.vector.reciprocal(out=ks8[:], in_=ks8[:])
                nc.vector.tensor_scalar_mul(out=kw[:], in0=kw[:], scalar1=ks8[:])
                acc = sb.tile([P, D], F32, tag="acc")
                for j in range(KK):
                    vg = gp.tile([P, D], BF16, tag="vg")
                    nc.gpsimd.indirect_dma_start(
                        out=vg[:], out_offset=None, in_=kV_in[:],
                        in_offset=bass.IndirectOffsetOnAxis(ap=idx[:, j:j + 1], axis=0))
                    if j == 0:
                        nc.vector.tensor_scalar(out=acc[:], in0=vg[:],
                                                scalar1=kw[:, 0:1], scalar2=None,
                                                op0=OP.mult)
                    else:
                        nc.vector.scalar_tensor_tensor(out=acc[:], in0=vg[:],
                                                       scalar=kw[:, j:j + 1],
                                                       in1=acc[:], op0=OP.mult,
                                                       op1=OP.add)
                outsb = sb.tile([P, D], F32, tag="outsb")
                nc.vector.tensor_tensor(out=outsb[:], in0=acc[:],
                                        in1=x_all[:, t, :], op=OP.add)
                nc.sync.dma_start(out=out_t[ts, :], in_=outsb[:])

    nc.finalize()
    return nc


@functools.lru_cache(maxsize=1)
def _get_program():
    return build_program()


def _prep_core_inputs(inputs):
    bf = ml_dtypes.bfloat16
    x = np.asarray(inputs["x"], np.float32)
    neurons = np.asarray(inputs["compress_neurons"], np.float32)
    neur_flat = np.ascontiguousarray(
        neurons.transpose(1, 0, 2).reshape(D, NCMP * R)).astype(bf)
    rqkv = np.concatenate([np.asarray(inputs["router_Q"], np.float32),
                           np.asarray(inputs["router_K"], np.float32),
                           np.asarray(inputs["router_V"], np.float32)],
                          axis=1).astype(bf)
    shared = dict(
        tri=np.where(np.tril(np.ones((P, P), bool)), 0.0, NEG).astype(np.float32),
        neurons=neur_flat,
        routersQKV=rqkv,
        routerM=np.asarray(inputs["router_M"], np.float32).astype(bf),
        W_Q=np.asarray(inputs["W_Q"], np.float32).astype(bf),
        W_K=np.asarray(inputs["W_K"], np.float32).astype(bf),
        W_V=np.asarray(inputs["W_V"], np.float32).astype(bf),
        W_O=np.asarray(inputs["W_O"], np.float32).astype(bf),
        kKT=np.ascontiguousarray(
            np.asarray(inputs["knowledge_K"], np.float32).T).astype(bf),
        kV=np.asarray(inputs["knowledge_V"], np.float32).astype(bf),
        ident=np.eye(P, dtype=np.float32).astype(bf),
    )
    in_maps = []
    for c in range(N_CORES):
        b, hf = c // 2, c % 2
        m = dict(shared)
        m["x_shard"] = np.ascontiguousarray(x[b, hf * TOK:(hf + 1) * TOK, :])
        m["offs"] = np.array([[hf * 2048, hf * 512]], np.uint32)
        in_maps.append(m)
    return in_maps


def kernel(**inputs) -> np.ndarray:
    nc = _get_program()
    in_maps = _prep_core_inputs(inputs)
    res = run_bass_kernel_spmd(nc, in_maps, list(range(N_CORES)))
    out = np.empty((B, S, D), np.float32)
    for c in range(N_CORES):
        b, hf = c // 2, c % 2
        out[b, hf * TOK:(hf + 1) * TOK, :] = res.results[c]["out_shard"]
    return out



# revision 5
# speedup vs baseline: 3.2719x; 3.2719x over previous
"""DAWNBlock Trainium2 kernel (8 NeuronCores, SPMD, single NEFF launch).

Sharding: tokens split over cores as (batch b = c//2, seq-half hf = c%2),
512 tokens per core. Attention is sharded by (batch, head-group): after a
pair AllGather of Q^T/K^T/V each core runs causal attention for 8 heads over
the full 1024-token sequence of its batch; a second pair AllGather exchanges
attn^T so each core projects (W_O) only its own 512 tokens.

The knowledge stage is expert-sharded to avoid replicating the big tables:
each core holds 1/8 of knowledge_K^T ([128, 4096] bf16) and knowledge_V
([4096, 1024] bf16). Qm^T is all-gathered (tiny) so every core scores all
4096 tokens against its shard, takes a local top-8 per token with the
hardware max8 instruction over packed floats (bf16 score in the high 16
bits, in-chunk column in the low bits), and all-gathers the packed top-8
candidate lists. From the 64 gathered candidates per token every core
derives identical softmax stats (max, 8th-largest threshold, masked Z),
weights its own surviving candidates, gathers its local V rows via
indirect DMA, and a ReduceScatter sums partial outputs back to the token
owners.

Core-parity-dependent data movement (which half of the pair AllGather output
belongs to this core) is handled with register-backed dynamic DMA slices
(bass.ds) driven by a tiny per-core offsets input, so all 8 cores share one
instruction stream.
"""
import functools
import numpy as np
import ml_dtypes

import concourse.bass as bass
import concourse.bacc as bacc
import concourse.mybir as mybir
import concourse.tile as tile
from concourse.bass_utils import run_bass_kernel_spmd

F32 = mybir.dt.float32
BF16 = mybir.dt.bfloat16
U32 = mybir.dt.uint32
U16 = mybir.dt.uint16
AF = mybir.ActivationFunctionType
OP = mybir.AluOpType
AX = mybir.AxisListType

N_CORES = 8
P = 128
D = 1024
R = 128
NCMP = 16
NK = 32768
NKS = NK // N_CORES    # 4096 knowledge rows per core
KK = 8
S = 1024
B = 4
TOK = 512
NT = TOK // P          # 4 token tiles per core
NTT = B * S // P       # 32 token tiles globally
EPS = 1e-5
NEG = -1.0e30
KC = 1024              # knowledge-score chunk width
NKC = NKS // KC        # 4 chunks per core shard
SCALE_R = float(1.0 / np.sqrt(R))


def _ln(nc, sb, x_ap, out_ap, eps_tile):
    """LayerNorm (gamma=1, beta=0): x_ap [128, D] f32 -> out_ap (bf16)."""
    stats = sb.tile([P, 2, 6], F32, tag="ln_stats")
    for g in range(2):
        nc.vector.bn_stats(out=stats[:, g, :], in_=x_ap[:, g * 512:(g + 1) * 512])
    mv = sb.tile([P, 2], F32, tag="ln_mv")
    nc.vector.bn_aggr(out=mv[:], in_=stats[:])
    rstd = sb.tile([P, 1], F32, tag="ln_rstd")
    nc.scalar.activation(out=rstd[:], in_=mv[:, 1:2], func=AF.Sqrt,
                         bias=eps_tile[:], scale=1.0)
    nc.vector.reciprocal(out=rstd[:], in_=rstd[:])
    nc.vector.tensor_scalar(out=out_ap, in0=x_ap, scalar1=mv[:, 0:1],
                            scalar2=rstd[:], op0=OP.subtract, op1=OP.mult)


def _softmax16(nc, sb, logits_ap, w_ap):
    """softmax over 16 router logits (PSUM f32 in) -> w_ap [128,16] f32."""
    mx = sb.tile([P, 1], F32, tag="rs_mx")
    nc.vector.tensor_reduce(out=mx[:], in_=logits_ap, axis=AX.X, op=OP.max)
    nmx = sb.tile([P, 1], F32, tag="rs_nmx")
    nc.vector.tensor_scalar_mul(out=nmx[:], in0=mx[:], scalar1=-1.0)
    ssum = sb.tile([P, 1], F32, tag="rs_sum")
    nc.scalar.activation(out=w_ap, in_=logits_ap, func=AF.Exp,
                         bias=nmx[:], scale=1.0, accum_out=ssum[:])
    nc.vector.reciprocal(out=ssum[:], in_=ssum[:])
    nc.vector.tensor_scalar_mul(out=w_ap, in0=w_ap, scalar1=ssum[:])


def _combine(nc, sb, p1_halves, w_ap, out_ap):
    """out[t,:] = sum_n w[t,n] * P1[t, n*128:(n+1)*128] (P1 in 2 PSUM halves)."""
    acc = sb.tile([P, R], F32, tag="cmb_acc")
    for n in range(NCMP):
        src = p1_halves[n // 8][:, (n % 8) * R:(n % 8 + 1) * R]
        if n == 0:
            nc.vector.tensor_scalar(out=acc[:], in0=src, scalar1=w_ap[:, 0:1],
                                    scalar2=None, op0=OP.mult)
        else:
            nc.vector.scalar_tensor_tensor(out=acc[:], in0=src,
                                           scalar=w_ap[:, n:n + 1], in1=acc[:],
                                           op0=OP.mult, op1=OP.add)
    nc.vector.tensor_copy(out=out_ap, in_=acc[:])


def build_program():
    nc = bacc.Bacc(None, num_devices=N_CORES)

    x_in = nc.dram_tensor("x_shard", [TOK, D], F32, kind="ExternalInput")
    tri_in = nc.dram_tensor("tri", [P, P], F32, kind="ExternalInput")
    neur_in = nc.dram_tensor("neurons", [D, NCMP * R], BF16, kind="ExternalInput")
    rQKV_in = nc.dram_tensor("routersQKV", [D, 48], BF16, kind="ExternalInput")
    rM_in = nc.dram_tensor("routerM", [D, NCMP], BF16, kind="ExternalInput")
    wq_in = nc.dram_tensor("W_Q", [R, D], BF16, kind="ExternalInput")
    wk_in = nc.dram_tensor("W_K", [R, D], BF16, kind="ExternalInput")
    wv_in = nc.dram_tensor("W_V", [R, D], BF16, kind="ExternalInput")
    wo_in = nc.dram_tensor("W_O", [D, D], BF16, kind="ExternalInput")
    kKT_in = nc.dram_tensor("kKT", [R, NKS], BF16, kind="ExternalInput")
    kV_in = nc.dram_tensor("kV", [NKS, D], BF16, kind="ExternalInput")
    id_in = nc.dram_tensor("ident", [P, P], BF16, kind="ExternalInput")
    offs_in = nc.dram_tensor("offs", [1, 2], U32, kind="ExternalInput")
    out_t = nc.dram_tensor("out_shard", [TOK, D], F32, kind="ExternalOutput")

    with tile.TileContext(nc) as tc:
        with (
            tc.tile_pool(name="persist", bufs=1) as pp,
            tc.tile_pool(name="weights", bufs=1) as wp,
            tc.tile_pool(name="work", bufs=2) as sb,
            tc.tile_pool(name="gath", bufs=3) as gp,
            tc.tile_pool(name="ps_big", bufs=2, space="PSUM") as psb,
            tc.tile_pool(name="ps_tp", bufs=2, space="PSUM") as pst,
            tc.tile_pool(name="ps_sm", bufs=2, space="PSUM") as psa,
            tc.tile_pool(name="dram", bufs=1, space="DRAM") as dram,
        ):
            # ---- parity offsets -> gpsimd registers for dynamic DMA slices ----
            r2048 = nc.gpsimd.alloc_register("off2048")
            nc.gpsimd.reg_load(r2048, offs_in[0:1, 0:1])
            off2048 = nc.gpsimd.snap(r2048, donate=True, min_val=0, max_val=2048)
            r512 = nc.gpsimd.alloc_register("off512")
            nc.gpsimd.reg_load(r512, offs_in[0:1, 1:2])
            off512 = nc.gpsimd.snap(r512, donate=True, min_val=0, max_val=512)

            # ---- resident weights ----
            neur = wp.tile([P, 8, NCMP * R], BF16)
            nc.sync.dma_start(out=neur[:], in_=neur_in[:].rearrange("(c p) n -> p c n", p=P))
            rQKV = wp.tile([P, 8, 48], BF16)
            nc.sync.dma_start(out=rQKV[:], in_=rQKV_in[:].rearrange("(c p) n -> p c n", p=P))
            rM = wp.tile([P, 8, NCMP], BF16)
            nc.sync.dma_start(out=rM[:], in_=rM_in[:].rearrange("(c p) n -> p c n", p=P))
            wq = wp.tile([P, D], BF16)
            nc.sync.dma_start(out=wq[:], in_=wq_in[:])
            wk = wp.tile([P, D], BF16)
            nc.sync.dma_start(out=wk[:], in_=wk_in[:])
            wv = wp.tile([P, D], BF16)
            nc.sync.dma_start(out=wv[:], in_=wv_in[:])
            wo = wp.tile([P, 8, D], BF16)
            nc.sync.dma_start(out=wo[:], in_=wo_in[:].rearrange("(c p) n -> p c n", p=P))
            ident = wp.tile([P, P], BF16)
            nc.sync.dma_start(out=ident[:], in_=id_in[:])
            tri = wp.tile([P, P], F32)
            nc.sync.dma_start(out=tri[:], in_=tri_in[:])
            kkt = wp.tile([P, NKS], BF16)
            nc.sync.dma_start(out=kkt[:], in_=kKT_in[:])
            eps_t = wp.tile([P, 1], F32)
            nc.vector.memset(eps_t[:], EPS)
            iota_t = wp.tile([P, KC], U16)
            nc.gpsimd.iota(out=iota_t[:], pattern=[[64, KC]], base=0,
                           channel_multiplier=0)

            # ---- persistent activations ----
            x_all = pp.tile([P, NT, D], F32)
            hT = pp.tile([P, 8, TOK], BF16, tag="hT")
            hQT = pp.tile([P, TOK], BF16, tag="hQT")
            hKT = pp.tile([P, TOK], BF16, tag="hKT")
            hVT = pp.tile([P, TOK], BF16, tag="hVT")
            QT_sb = pp.tile([P, 8, TOK], BF16, tag="qt")
            KT_sb = pp.tile([P, 8, TOK], BF16, tag="kt")
            V_sb = pp.tile([P, NT, D], BF16, tag="vv")

            # packed score buffers (iota pre-written into the low u16 lanes)
            packed = [pp.tile([P, KC], U32, tag=f"pk{i}", name=f"pk{i}") for i in range(3)]
            for pk in packed:
                nc.vector.tensor_copy(out=pk.bitcast(U16)[:, 0::2], in_=iota_t[:])

            # =========== S1: LN1, shared projection, routed compress ===========
            for t in range(NT):
                ts = slice(t * P, (t + 1) * P)
                nc.sync.dma_start(out=x_all[:, t, :], in_=x_in[ts, :])
                h = sb.tile([P, D], BF16, tag="h")
                _ln(nc, sb, x_all[:, t, :], h[:], eps_t)
                for ch in range(8):
                    tp = pst.tile([P, P], BF16, tag="tp")
                    nc.tensor.transpose(out=tp[:], in_=h[:, ch * P:(ch + 1) * P],
                                        identity=ident[:])
                    nc.scalar.activation(out=hT[:, ch, ts], in_=tp[:], func=AF.Copy)
                lg = psa.tile([P, 48], F32, tag="sm")
                for ch in range(8):
                    nc.tensor.matmul(out=lg[:], lhsT=hT[:, ch, ts], rhs=rQKV[:, ch, :],
                                     start=(ch == 0), stop=(ch == 7))
                wQKV = sb.tile([P, 48], F32, tag="wQKV")
                for rr in range(3):
                    _softmax16(nc, sb, lg[:, rr * 16:(rr + 1) * 16],
                               wQKV[:, rr * 16:(rr + 1) * 16])
                p1a = psb.tile([P, KC], F32, tag="big")
                p1b = psb.tile([P, KC], F32, tag="big")
                for half, pt in ((0, p1a), (1, p1b)):
                    for col in range(2):
                        c0 = half * KC + col * 512
                        for ch in range(8):
                            nc.tensor.matmul(out=pt[:, col * 512:(col + 1) * 512],
                                             lhsT=hT[:, ch, ts],
                                             rhs=neur[:, ch, c0:c0 + 512],
                                             start=(ch == 0), stop=(ch == 7))
                for rr, dst in ((0, hQT), (1, hKT), (2, hVT)):
                    hc = sb.tile([P, R], BF16, tag="hc")
                    _combine(nc, sb, (p1a, p1b), wQKV[:, rr * 16:(rr + 1) * 16], hc[:])
                    tp = pst.tile([P, P], BF16, tag="tp")
                    nc.tensor.transpose(out=tp[:], in_=hc[:], identity=ident[:])
                    nc.scalar.activation(out=dst[:, ts], in_=tp[:], func=AF.Copy)

            # =========== S2: Q^T / K^T (all 16 heads) and V ===========
            for ch in range(8):
                for w_, hsrc, dst in ((wq, hQT, QT_sb), (wk, hKT, KT_sb)):
                    pr = pst.tile([P, TOK], F32, tag="tp")
                    nc.tensor.matmul(out=pr[:], lhsT=w_[:, ch * P:(ch + 1) * P],
                                     rhs=hsrc[:], start=True, stop=True)
                    nc.scalar.activation(out=dst[:, ch, :], in_=pr[:], func=AF.Copy)
            for t in range(NT):
                pv = psb.tile([P, D], F32, tag="big")
                for col in range(2):
                    nc.tensor.matmul(out=pv[:, col * 512:(col + 1) * 512],
                                     lhsT=hVT[:, t * P:(t + 1) * P],
                                     rhs=wv[:, col * 512:(col + 1) * 512],
                                     start=True, stop=True)
                nc.scalar.activation(out=V_sb[:, t, :], in_=pv[:], func=AF.Copy)

            # =========== S3: pair AllGather of QT/KT/V ===========
            groups = [[0, 1], [2, 3], [4, 5], [6, 7]]
            group8 = [list(range(N_CORES))]
            xin = dram.tile([P, 12288], BF16)
            xout = dram.tile([2 * P, 12288], BF16)
            nc.gpsimd.dma_start(out=xin[:, 0:4096],
                                in_=QT_sb[:].rearrange("p c t -> p (c t)"))
            nc.gpsimd.dma_start(out=xin[:, 4096:8192],
                                in_=KT_sb[:].rearrange("p c t -> p (c t)"))
            nc.gpsimd.dma_start(out=xin[:, 8192:12288],
                                in_=V_sb[:].rearrange("p c t -> p (c t)"))
            nc.gpsimd.collective_compute("AllGather", OP.bypass,
                                         replica_groups=groups,
                                         ins=[xin.opt()], outs=[xout.opt()])
            # reuse the big persistent slots for the assembled full-seq tensors
            QT_f = pp.tile([P, 4, S], BF16, tag="qt")
            KT_f = pp.tile([P, 4, S], BF16, tag="kt")
            V_f = pp.tile([P, 8, 512], BF16, tag="vv")
            for src in range(2):
                rs = slice(src * P, (src + 1) * P)
                qsl = slice(src * TOK, (src + 1) * TOK)
                for i in range(4):
                    nc.gpsimd.dma_start(
                        out=QT_f[:, i, qsl],
                        in_=xout[rs, 0:4096][:, bass.ds(off2048 + i * TOK, TOK)])
                    nc.gpsimd.dma_start(
                        out=KT_f[:, i, qsl],
                        in_=xout[rs, 4096:8192][:, bass.ds(off2048 + i * TOK, TOK)])
                    nc.gpsimd.dma_start(
                        out=V_f[:, src * 4 + i, :],
                        in_=xout[rs, 8192:12288][:, bass.ds(off512 + i * D, 512)])

            # =========== S4: causal attention, 8 heads, full sequence ===========
            attnT = pp.tile([P, 4, S], BF16, tag="at")
            for hh in range(8):
                ch, poff = hh // 2, (hh % 2) * 64
                prow = slice(poff, poff + 64)
                for qg in range(8):
                    kr = (qg + 1) * P
                    sc = psb.tile([P, S], F32, tag="big")
                    for part in range((kr + 511) // 512):
                        k0, k1 = part * 512, min(kr, (part + 1) * 512)
                        nc.tensor.matmul(out=sc[:, k0:k1],
                                         lhsT=QT_f[prow, ch, qg * P:(qg + 1) * P],
                                         rhs=KT_f[prow, ch, k0:k1],
                                         start=True, stop=True)
                    mtmp = sb.tile([P, P], F32, tag="mtmp")
                    nc.vector.tensor_tensor(out=mtmp[:], in0=sc[:, qg * P:kr],
                                            in1=tri[:], op=OP.add)
                    Pb = sb.tile([P, S], BF16, tag="Pb")
                    s2 = sb.tile([P, 1], F32, tag="s2")
                    if qg > 0:
                        s1 = sb.tile([P, 1], F32, tag="s1")
                        nc.scalar.activation(out=Pb[:, 0:qg * P], in_=sc[:, 0:qg * P],
                                             func=AF.Exp, scale=0.125, accum_out=s1[:])
                    nc.scalar.activation(out=Pb[:, qg * P:kr], in_=mtmp[:],
                                         func=AF.Exp, scale=0.125, accum_out=s2[:])
                    den = sb.tile([P, 1], F32, tag="den")
                    if qg > 0:
                        nc.vector.tensor_tensor(out=den[:], in0=s1[:], in1=s2[:],
                                                op=OP.add)
                    else:
                        nc.vector.tensor_copy(out=den[:], in_=s2[:])
                    nc.vector.reciprocal(out=den[:], in_=den[:])
                    diag = sb.tile([P, P], BF16, tag="diag")
                    nc.vector.tensor_tensor(out=diag[:], in0=ident[:],
                                            in1=den[:].to_broadcast([P, P]),
                                            op=OP.mult)
                    at = psa.tile([64, P], F32, tag="sm")
                    for kb in range(qg + 1):
                        ptp = pst.tile([P, P], F32, tag="tp")
                        nc.tensor.matmul(out=ptp[:],
                                         lhsT=Pb[:, kb * P:(kb + 1) * P],
                                         rhs=diag[:], start=True, stop=True)
                        pts = sb.tile([P, P], BF16, tag="pts")
                        nc.scalar.activation(out=pts[:], in_=ptp[:], func=AF.Copy)
                        nc.tensor.matmul(out=at[:],
                                         lhsT=V_f[:, kb, hh * 64:(hh + 1) * 64],
                                         rhs=pts[:], start=(kb == 0), stop=(kb == qg))
                    nc.scalar.activation(out=attnT[prow, ch, qg * P:(qg + 1) * P],
                                         in_=at[:], func=AF.Copy)

            # =========== S5: exchange attn^T, W_O, residual ===========
            xin2 = dram.tile([P, 4 * S], BF16)
            xout2 = dram.tile([2 * P, 4 * S], BF16)
            nc.gpsimd.dma_start(out=xin2[:], in_=attnT[:].rearrange("p c q -> p (c q)"))
            nc.gpsimd.collective_compute("AllGather", OP.bypass,
                                         replica_groups=groups,
                                         ins=[xin2.opt()], outs=[xout2.opt()])
            aT = pp.tile([P, 8, TOK], BF16, tag="at")
            for src in range(2):
                rs = slice(src * P, (src + 1) * P)
                for i in range(4):
                    nc.gpsimd.dma_start(
                        out=aT[:, src * 4 + i, :],
                        in_=xout2[rs, :][:, bass.ds(off512 + i * S, TOK)])
            for t in range(NT):
                ts = slice(t * P, (t + 1) * P)
                po = psb.tile([P, D], F32, tag="big")
                for col in range(2):
                    for ch in range(8):
                        nc.tensor.matmul(out=po[:, col * 512:(col + 1) * 512],
                                         lhsT=aT[:, ch, ts],
                                         rhs=wo[:, ch, col * 512:(col + 1) * 512],
                                         start=(ch == 0), stop=(ch == 7))
                nc.vector.tensor_tensor(out=x_all[:, t, :], in0=po[:],
                                        in1=x_all[:, t, :], op=OP.add)

            # =========== S6: LN2 + compress M -> Qm^T (into hQT) ===========
            for t in range(NT):
                ts = slice(t * P, (t + 1) * P)
                h2 = sb.tile([P, D], BF16, tag="h")
                _ln(nc, sb, x_all[:, t, :], h2[:], eps_t)
                for ch in range(8):
                    tp = pst.tile([P, P], BF16, tag="tp")
                    nc.tensor.transpose(out=tp[:], in_=h2[:, ch * P:(ch + 1) * P],
                                        identity=ident[:])
                    nc.scalar.activation(out=hT[:, ch, ts], in_=tp[:], func=AF.Copy)
                lgm = psa.tile([P, NCMP], F32, tag="sm")
                for ch in range(8):
                    nc.tensor.matmul(out=lgm[:], lhsT=hT[:, ch, ts], rhs=rM[:, ch, :],
                                     start=(ch == 0), stop=(ch == 7))
                wM = sb.tile([P, NCMP], F32, tag="wM")
                _softmax16(nc, sb, lgm[:], wM[:])
                p1a = psb.tile([P, KC], F32, tag="big")
                p1b = psb.tile([P, KC], F32, tag="big")
                for half, pt in ((0, p1a), (1, p1b)):
                    for col in range(2):
                        c0 = half * KC + col * 512
                        for ch in range(8):
                            nc.tensor.matmul(out=pt[:, col * 512:(col + 1) * 512],
                                             lhsT=hT[:, ch, ts],
                                             rhs=neur[:, ch, c0:c0 + 512],
                                             start=(ch == 0), stop=(ch == 7))
                qm = sb.tile([P, R], BF16, tag="hc")
                _combine(nc, sb, (p1a, p1b), wM[:], qm[:])
                tp = pst.tile([P, P], BF16, tag="tp")
                nc.tensor.transpose(out=tp[:], in_=qm[:], identity=ident[:])
                nc.scalar.activation(out=hQT[:, ts], in_=tp[:], func=AF.Copy)

            # =========== S7a: AllGather Qm^T across all 8 cores ===========
            xin3 = dram.tile([P, TOK], BF16)
            xout3 = dram.tile([N_CORES * P, TOK], BF16)
            nc.gpsimd.dma_start(out=xin3[:], in_=hQT[:])
            nc.gpsimd.collective_compute("AllGather", OP.bypass,
                                         replica_groups=group8,
                                         ins=[xin3.opt()], outs=[xout3.opt()])
            QmT_f = pp.tile([P, N_CORES, TOK], BF16, tag="qt")
            for c in range(N_CORES):
                nc.gpsimd.dma_start(out=QmT_f[:, c, :],
                                    in_=xout3[c * P:(c + 1) * P, :])

            # ===== S7b: scores vs local shard + local top-8, all 32 tiles =====
            cands = pp.tile([P, NTT, NKC * 8], U32, tag="cands")
            top8a = pp.tile([P, NTT, 8], U32, tag="top8a")
            for q in range(NTT):
                lq = QmT_f[:, q // 4, (q % 4) * P:(q % 4 + 1) * P]
                for ch in range(NKC):
                    ks = psb.tile([P, KC], F32, tag="big")
                    for col in range(2):
                        c0 = ch * KC + col * 512
                        nc.tensor.matmul(out=ks[:, col * 512:(col + 1) * 512],
                                         lhsT=lq, rhs=kkt[:, c0:c0 + 512],
                                         start=True, stop=True)
                    pk = packed[(q * NKC + ch) % 3]
                    nc.scalar.activation(out=pk.bitcast(U16)[:, 1::2].bitcast(BF16),
                                         in_=ks[:], func=AF.Copy)
                    c8 = cands[:, q, ch * 8:(ch + 1) * 8]
                    nc.vector.max(out=c8.bitcast(F32), in_=pk.bitcast(F32)[:])
                t8 = top8a[:, q, :]
                nc.vector.max(out=t8.bitcast(F32), in_=cands.bitcast(F32)[:, q, :])

            # =========== S7c: AllGather packed top-8 candidates ===========
            xin4 = dram.tile([P, NTT * 8], U32)
            xout4 = dram.tile([N_CORES * P, NTT * 8], U32)
            nc.gpsimd.dma_start(out=xin4[:],
                                in_=top8a[:].rearrange("p t s -> p (t s)"))
            nc.gpsimd.collective_compute("AllGather", OP.bypass,
                                         replica_groups=group8,
                                         ins=[xin4.opt()], outs=[xout4.opt()])
            cand_all = pp.tile([P, NTT, N_CORES * 8], U32, tag="hT")
            for c in range(N_CORES):
                nc.sync.dma_start(
                    out=cand_all[:, :, c * 8:(c + 1) * 8],
                    in_=xout4[c * P:(c + 1) * P, :].rearrange("p (t s) -> p t s", s=8))

            # ==== S7d: per-token softmax stats, my weights, decode my idx ====
            m8_all = pp.tile([P, NTT, 8], F32, tag="m8a")
            for q in range(NTT):
                nc.vector.max(out=m8_all[:, q, :], in_=cand_all.bitcast(F32)[:, q, :])
            # all-candidate scores, exp, threshold mask, Z
            # (scores are tiny, |s| < 1, so the usual max-subtraction before
            # exp is unnecessary; softmax is shift-invariant)
            s_all = pp.tile([P, NTT, N_CORES * 8], F32, tag="kt")
            nc.vector.tensor_scalar(out=s_all[:].bitcast(U32), in0=cand_all[:],
                                    scalar1=0xFFFF0000, scalar2=None,
                                    op0=OP.bitwise_and)
            ex_all = pp.tile([P, NTT, N_CORES * 8], F32, tag="vv")
            nc.scalar.activation(out=ex_all[:], in_=s_all[:], func=AF.Exp,
                                 scale=SCALE_R)
            mask_all = pp.tile([P, NTT, N_CORES * 8], F32, tag="mska")
            nc.vector.tensor_tensor(out=mask_all[:], in0=cand_all.bitcast(F32)[:],
                                    in1=m8_all[:, :, 7:8].to_broadcast(
                                        [P, NTT, N_CORES * 8]),
                                    op=OP.is_ge)
            nc.vector.tensor_tensor(out=ex_all[:], in0=ex_all[:], in1=mask_all[:],
                                    op=OP.mult)
            zz = pp.tile([P, NTT, 1], F32, tag="zz")
            nc.vector.tensor_reduce(out=zz[:], in_=ex_all[:], axis=AX.X, op=OP.add)
            nc.vector.reciprocal(out=zz[:], in_=zz[:])
            # my candidates: scores, exp, mask, weights
            s8a = pp.tile([P, NTT, 8], F32, tag="s8a")
            nc.vector.tensor_scalar(out=s8a[:].bitcast(U32), in0=top8a[:],
                                    scalar1=0xFFFF0000, scalar2=None,
                                    op0=OP.bitwise_and)
            w8_all = pp.tile([P, NTT, 8], F32, tag="w8a")
            nc.scalar.activation(out=w8_all[:], in_=s8a[:], func=AF.Exp,
                                 scale=SCALE_R)
            msk8 = pp.tile([P, NTT, 8], F32, tag="msk8")
            nc.vector.tensor_tensor(out=msk8[:], in0=top8a.bitcast(F32)[:],
                                    in1=m8_all[:, :, 7:8].to_broadcast([P, NTT, 8]),
                                    op=OP.is_ge)
            nc.vector.tensor_tensor(out=w8_all[:], in0=w8_all[:], in1=msk8[:],
                                    op=OP.mult)
            nc.vector.tensor_tensor(out=w8_all[:], in0=w8_all[:],
                                    in1=zz[:].to_broadcast([P, NTT, 8]),
                                    op=OP.mult)
            # decode my local knowledge-row indices
            pos_all = pp.tile([P, NTT, 8], U32, tag="posa")
            for q in range(NTT):
                nc.vector.max_index(out=pos_all[:, q, :],
                                    in_max=top8a.bitcast(F32)[:, q, :],
                                    in_values=cands.bitcast(F32)[:, q, :])
            idx_all = pp.tile([P, NTT, 8], U32, tag="idxa")
            nc.vector.tensor_scalar(out=idx_all[:], in0=pos_all[:],
                                    scalar1=3, scalar2=10,
                                    op0=OP.logical_shift_right,
                                    op1=OP.logical_shift_left)
            loc_all = pp.tile([P, NTT, 8], U32, tag="loca")
            nc.vector.tensor_scalar(out=loc_all[:], in0=top8a[:],
                                    scalar1=6, scalar2=0x3FF,
                                    op0=OP.logical_shift_right,
                                    op1=OP.bitwise_and)
            nc.vector.tensor_tensor(out=idx_all[:], in0=idx_all[:], in1=loc_all[:],
                                    op=OP.bitwise_or)

            # ==== S7e: gather my V rows, weighted partials, ReduceScatter ====
            rsin = dram.tile([NTT * P, D], BF16)
            rsout = dram.tile([NT * P, D], BF16)
            for q in range(NTT):
                acc = sb.tile([P, D], F32, tag="acc")
                for j in range(KK):
                    vg = gp.tile([P, D], BF16, tag="vg")
                    nc.gpsimd.indirect_dma_start(
                        out=vg[:], out_offset=None, in_=kV_in[:],
                        in_offset=bass.IndirectOffsetOnAxis(
                            ap=idx_all[:, q, j:j + 1], axis=0))
                    if j == 0:
                        nc.vector.tensor_scalar(out=acc[:], in0=vg[:],
                                                scalar1=w8_all[:, q, 0:1],
                                                scalar2=None, op0=OP.mult)
                    else:
                        nc.vector.scalar_tensor_tensor(out=acc[:], in0=vg[:],
                                                       scalar=w8_all[:, q, j:j + 1],
                                                       in1=acc[:], op0=OP.mult,
                                                       op1=OP.add)
                accb = sb.tile([P, D], BF16, tag="accb")
                nc.scalar.activation(out=accb[:], in_=acc[:], func=AF.Copy)
                nc.sync.dma_start(out=rsin[q * P:(q + 1) * P, :], in_=accb[:])
            nc.gpsimd.collective_compute("ReduceScatter", OP.add,
                                         replica_groups=group8,
                                         ins=[rsin.opt()], outs=[rsout.opt()])
            for t in range(NT):
                ts = slice(t * P, (t + 1) * P)
                mem = gp.tile([P, D], BF16, tag="vg")
                nc.sync.dma_start(out=mem[:], in_=rsout[t * P:(t + 1) * P, :])
                outsb = sb.tile([P, D], F32, tag="outsb")
                nc.vector.tensor_tensor(out=outsb[:], in0=mem[:],
                                        in1=x_all[:, t, :], op=OP.add)
                nc.sync.dma_start(out=out_t[ts, :], in_=outsb[:])

    nc.finalize()
    return nc


@functools.lru_cache(maxsize=1)
def _get_program():
    return build_program()


def _prep_core_inputs(inputs):
    bf = ml_dtypes.bfloat16
    x = np.asarray(inputs["x"], np.float32)
    neurons = np.asarray(inputs["compress_neurons"], np.float32)
    neur_flat = np.ascontiguousarray(
        neurons.transpose(1, 0, 2).reshape(D, NCMP * R)).astype(bf)
    rqkv = np.concatenate([np.asarray(inputs["router_Q"], np.float32),
                           np.asarray(inputs["router_K"], np.float32),
                           np.asarray(inputs["router_V"], np.float32)],
                          axis=1).astype(bf)
    kKT_full = np.ascontiguousarray(
        np.asarray(inputs["knowledge_K"], np.float32).T).astype(bf)
    kV_full = np.asarray(inputs["knowledge_V"], np.float32).astype(bf)
    shared = dict(
        tri=np.where(np.tril(np.ones((P, P), bool)), 0.0, NEG).astype(np.float32),
        neurons=neur_flat,
        routersQKV=rqkv,
        routerM=np.asarray(inputs["router_M"], np.float32).astype(bf),
        W_Q=np.asarray(inputs["W_Q"], np.float32).astype(bf),
        W_K=np.asarray(inputs["W_K"], np.float32).astype(bf),
        W_V=np.asarray(inputs["W_V"], np.float32).astype(bf),
        W_O=np.asarray(inputs["W_O"], np.float32).astype(bf),
        ident=np.eye(P, dtype=np.float32).astype(bf),
    )
    in_maps = []
    for c in range(N_CORES):
        b, hf = c // 2, c % 2
        m = dict(shared)
        m["x_shard"] = np.ascontiguousarray(x[b, hf * TOK:(hf + 1) * TOK, :])
        m["offs"] = np.array([[hf * 2048, hf * 512]], np.uint32)
        m["kKT"] = np.ascontiguousarray(kKT_full[:, c * NKS:(c + 1) * NKS])
        m["kV"] = np.ascontiguousarray(kV_full[c * NKS:(c + 1) * NKS, :])
        in_maps.append(m)
    return in_maps


def kernel(**inputs) -> np.ndarray:
    nc = _get_program()
    in_maps = _prep_core_inputs(inputs)
    res = run_bass_kernel_spmd(nc, in_maps, list(range(N_CORES)))
    out = np.empty((B, S, D), np.float32)
    for c in range(N_CORES):
        b, hf = c // 2, c % 2
        out[b, hf * TOK:(hf + 1) * TOK, :] = res.results[c]["out_shard"]
    return out


# revision 14
# speedup vs baseline: 6.4757x; 1.9792x over previous
"""DAWNBlock Trainium2 kernel (8 NeuronCores, SPMD, single NEFF launch).

Sharding: tokens split over cores as (batch b = c//2, seq-half hf = c%2),
512 tokens per core. Attention is sharded by (batch, head-group): after a
pair AllGather of Q^T/K^T/V each core runs causal attention for 8 heads over
the full 1024-token sequence of its batch; a second pair AllGather exchanges
attn^T so each core projects (W_O) only its own 512 tokens.

The knowledge stage is expert-sharded to avoid replicating the big tables:
each core holds 1/8 of knowledge_K^T ([128, 4096] bf16) and knowledge_V
([4096, 1024] bf16). Qm^T is all-gathered (tiny) so every core scores all
4096 tokens against its shard, takes a local top-8 per token with the
hardware max8 instruction over packed floats (bf16 score in the high 16
bits, in-chunk column in the low bits), and all-gathers the packed top-8
candidate lists. From the 64 gathered candidates per token every core
derives identical softmax stats (max, 8th-largest threshold, masked Z),
weights its own surviving candidates, gathers its local V rows via
indirect DMA, and a ReduceScatter sums partial outputs back to the token
owners.

Core-parity-dependent data movement (which half of the pair AllGather output
belongs to this core) is handled with register-backed dynamic DMA slices
(bass.ds) driven by a tiny per-core offsets input, so all 8 cores share one
instruction stream.
"""
import functools
import numpy as np
import ml_dtypes

import concourse.bass as bass
import concourse.bacc as bacc
import concourse.mybir as mybir
import concourse.tile as tile
from concourse.bass_utils import run_bass_kernel_spmd

F32 = mybir.dt.float32
BF16 = mybir.dt.bfloat16
U32 = mybir.dt.uint32
U16 = mybir.dt.uint16
AF = mybir.ActivationFunctionType
OP = mybir.AluOpType
AX = mybir.AxisListType

N_CORES = 8
P = 128
D = 1024
R = 128
NCMP = 16
NK = 32768
NKS = NK // N_CORES    # 4096 knowledge rows per core
KK = 8
S = 1024
B = 4
TOK = 512
NT = TOK // P          # 4 token tiles per core
NTT = B * S // P       # 32 token tiles globally
EPS = 1e-5
NEG = -1.0e30
KC = 1024              # knowledge-score chunk width
NKC = NKS // KC        # 4 chunks per core shard
SCALE_R = float(1.0 / np.sqrt(R))


def _ln(nc, sb, x_ap, out_ap, eps_tile):
    """LayerNorm (gamma=1, beta=0): x_ap [128, D] f32 -> out_ap (bf16)."""
    stats = sb.tile([P, 2, 6], F32, tag="ln_stats")
    for g in range(2):
        nc.vector.bn_stats(out=stats[:, g, :], in_=x_ap[:, g * 512:(g + 1) * 512])
    mv = sb.tile([P, 2], F32, tag="ln_mv")
    nc.vector.bn_aggr(out=mv[:], in_=stats[:])
    rstd = sb.tile([P, 1], F32, tag="ln_rstd")
    nc.scalar.activation(out=rstd[:], in_=mv[:, 1:2], func=AF.Sqrt,
                         bias=eps_tile[:], scale=1.0)
    nc.vector.reciprocal(out=rstd[:], in_=rstd[:])
    nc.vector.tensor_scalar(out=out_ap, in0=x_ap, scalar1=mv[:, 0:1],
                            scalar2=rstd[:], op0=OP.subtract, op1=OP.mult)


def _softmax16(nc, sb, logits_ap, w_ap):
    """softmax over 16 router logits (PSUM f32 in) -> w_ap [128,16] f32."""
    mx = sb.tile([P, 1], F32, tag="rs_mx")
    nc.vector.tensor_reduce(out=mx[:], in_=logits_ap, axis=AX.X, op=OP.max)
    nmx = sb.tile([P, 1], F32, tag="rs_nmx")
    nc.vector.tensor_scalar_mul(out=nmx[:], in0=mx[:], scalar1=-1.0)
    ssum = sb.tile([P, 1], F32, tag="rs_sum")
    nc.scalar.activation(out=w_ap, in_=logits_ap, func=AF.Exp,
                         bias=nmx[:], scale=1.0, accum_out=ssum[:])
    nc.vector.reciprocal(out=ssum[:], in_=ssum[:])
    nc.vector.tensor_scalar_mul(out=w_ap, in0=w_ap, scalar1=ssum[:])


def _combine(nc, sb, p1_halves, w_ap, out_ap):
    """out[t,:] = sum_n w[t,n] * P1[t, n*128:(n+1)*128] (P1 in 2 PSUM halves)."""
    acc = sb.tile([P, R], F32, tag="cmb_acc")
    for n in range(NCMP):
        src = p1_halves[n // 8][:, (n % 8) * R:(n % 8 + 1) * R]
        if n == 0:
            nc.vector.tensor_scalar(out=acc[:], in0=src, scalar1=w_ap[:, 0:1],
                                    scalar2=None, op0=OP.mult)
        else:
            nc.vector.scalar_tensor_tensor(out=acc[:], in0=src,
                                           scalar=w_ap[:, n:n + 1], in1=acc[:],
                                           op0=OP.mult, op1=OP.add)
    nc.vector.tensor_copy(out=out_ap, in_=acc[:])


def build_program():
    nc = bacc.Bacc(None, num_devices=N_CORES)

    x_in = nc.dram_tensor("x_shard", [TOK, D], BF16, kind="ExternalInput")
    neur_in = nc.dram_tensor("neur_sh", [D // 8, NCMP * R], BF16, kind="ExternalInput")
    rt_in = nc.dram_tensor("rt_sh", [D // 8, 64], BF16, kind="ExternalInput")
    wqkv_in = nc.dram_tensor("wqkv_sh", [48, D], BF16, kind="ExternalInput")
    wo_in = nc.dram_tensor("wo_sh", [D // 8, D], BF16, kind="ExternalInput")
    kKT_in = nc.dram_tensor("kKT", [R, NKS], BF16, kind="ExternalInput")
    kV_in = nc.dram_tensor("kV", [NKS, D], BF16, kind="ExternalInput")
    offs_in = nc.dram_tensor("offs", [1, 2], U32, kind="ExternalInput")
    out_t = nc.dram_tensor("out_shard", [TOK, D], BF16, kind="ExternalOutput")

    with tile.TileContext(nc) as tc:
        with (
            tc.tile_pool(name="persist", bufs=1) as pp,
            tc.tile_pool(name="weights", bufs=1) as wp,
            tc.tile_pool(name="work", bufs=2) as sb,
            tc.tile_pool(name="gath", bufs=3) as gp,
            tc.tile_pool(name="ps_big", bufs=2, space="PSUM") as psb,
            tc.tile_pool(name="ps_tp", bufs=2, space="PSUM") as pst,
            tc.tile_pool(name="ps_sm", bufs=2, space="PSUM") as psa,
            tc.tile_pool(name="dram", bufs=1, space="DRAM") as dram,
        ):
            # ---- parity offsets -> gpsimd registers for dynamic DMA slices ----
            r2048 = nc.gpsimd.alloc_register("off2048")
            nc.gpsimd.reg_load(r2048, offs_in[0:1, 0:1])
            off2048 = nc.gpsimd.snap(r2048, donate=True, min_val=0, max_val=2048)
            r512 = nc.gpsimd.alloc_register("off512")
            nc.gpsimd.reg_load(r512, offs_in[0:1, 1:2])
            off512 = nc.gpsimd.snap(r512, donate=True, min_val=0, max_val=512)

            group8 = [list(range(N_CORES))]

            # ---- broadcast replicated weights on-device (1/8 shard shipped
            # from host per core; AllGather along rows reassembles the full
            # tensor in rank==row-block order) ----
            stgN = dram.tile([D // 8, NCMP * R], BF16)
            nc.gpsimd.dma_start(out=stgN[:], in_=neur_in[:])
            agN = dram.tile([D, NCMP * R], BF16)
            nc.gpsimd.collective_compute("AllGather", OP.bypass,
                                         replica_groups=group8,
                                         ins=[stgN.opt()], outs=[agN.opt()])
            stgR = dram.tile([D // 8, 64], BF16)
            nc.gpsimd.dma_start(out=stgR[:], in_=rt_in[:])
            agR = dram.tile([D, 64], BF16)
            nc.gpsimd.collective_compute("AllGather", OP.bypass,
                                         replica_groups=group8,
                                         ins=[stgR.opt()], outs=[agR.opt()])
            stgW = dram.tile([48, D], BF16)
            nc.gpsimd.dma_start(out=stgW[:], in_=wqkv_in[:])
            agW = dram.tile([384, D], BF16)
            nc.gpsimd.collective_compute("AllGather", OP.bypass,
                                         replica_groups=group8,
                                         ins=[stgW.opt()], outs=[agW.opt()])
            stgO = dram.tile([D // 8, D], BF16)
            nc.gpsimd.dma_start(out=stgO[:], in_=wo_in[:])
            agO = dram.tile([D, D], BF16)
            nc.gpsimd.collective_compute("AllGather", OP.bypass,
                                         replica_groups=group8,
                                         ins=[stgO.opt()], outs=[agO.opt()])

            # ---- resident weights ----
            neur = wp.tile([P, 8, NCMP * R], BF16)
            nc.sync.dma_start(out=neur[:], in_=agN[:].rearrange("(c p) n -> p c n", p=P))
            rQKV = wp.tile([P, 8, 48], BF16)
            nc.sync.dma_start(out=rQKV[:],
                              in_=agR[:, 0:48].rearrange("(c p) n -> p c n", p=P))
            rM = wp.tile([P, 8, NCMP], BF16)
            nc.sync.dma_start(out=rM[:],
                              in_=agR[:, 48:64].rearrange("(c p) n -> p c n", p=P))
            wq = wp.tile([P, D], BF16)
            wk = wp.tile([P, D], BF16)
            wv = wp.tile([P, D], BF16)
            for c in range(N_CORES):
                for w_i, w_t in enumerate((wq, wk, wv)):
                    nc.sync.dma_start(
                        out=w_t[c * 16:(c + 1) * 16, :],
                        in_=agW[c * 48 + w_i * 16:c * 48 + (w_i + 1) * 16, :])
            wo = wp.tile([P, 8, D], BF16)
            nc.sync.dma_start(out=wo[:], in_=agO[:].rearrange("(c p) n -> p c n", p=P))
            kkt = wp.tile([P, NKS], BF16)
            nc.sync.dma_start(out=kkt[:], in_=kKT_in[:])
            eps_t = wp.tile([P, 1], F32)
            nc.vector.memset(eps_t[:], EPS)
            iota_t = wp.tile([P, KC], U16)
            nc.gpsimd.iota(out=iota_t[:], pattern=[[64, KC]], base=0,
                           channel_multiplier=0)

            # ---- generate ident (bf16 I) and tri (0 / -1e30 causal) ----
            colx = wp.tile([P, P], F32)
            nc.gpsimd.iota(out=colx[:], pattern=[[1, P]], base=0,
                           channel_multiplier=0,
                           allow_small_or_imprecise_dtypes=True)
            rowx = wp.tile([P, 1], F32)
            nc.gpsimd.iota(out=rowx[:], pattern=[[0, 1]], base=0,
                           channel_multiplier=1,
                           allow_small_or_imprecise_dtypes=True)
            ident = wp.tile([P, P], BF16)
            nc.vector.tensor_scalar(out=ident[:], in0=colx[:], scalar1=rowx[:],
                                    scalar2=None, op0=OP.is_equal)
            tri = wp.tile([P, P], F32)
            nc.vector.tensor_scalar(out=tri[:], in0=colx[:], scalar1=rowx[:],
                                    scalar2=NEG, op0=OP.is_gt, op1=OP.mult)

            # ---- persistent activations ----
            x_all = pp.tile([P, NT, D], F32)
            hT = pp.tile([P, 8, TOK], BF16, tag="hT")
            hQT = pp.tile([P, TOK], BF16, tag="hQT")
            hKT = pp.tile([P, TOK], BF16, tag="hKT")
            hVT = pp.tile([P, TOK], BF16, tag="hVT")
            QT_sb = pp.tile([P, 8, TOK], BF16, tag="qt")
            KT_sb = pp.tile([P, 8, TOK], BF16, tag="kt")
            V_sb = pp.tile([P, NT, D], BF16, tag="vv")

            # packed score buffers (iota pre-written into the low u16 lanes)
            packed = [pp.tile([P, KC], U32, tag=f"pk{i}", name=f"pk{i}") for i in range(3)]
            for pk in packed:
                nc.vector.tensor_copy(out=pk.bitcast(U16)[:, 0::2], in_=iota_t[:])

            # =========== S1: LN1, shared projection, routed compress ===========
            for t in range(NT):
                ts = slice(t * P, (t + 1) * P)
                xb = sb.tile([P, D], BF16, tag="xb")
                nc.sync.dma_start(out=xb[:], in_=x_in[ts, :])
                nc.vector.tensor_copy(out=x_all[:, t, :], in_=xb[:])
                h = sb.tile([P, D], BF16, tag="h")
                _ln(nc, sb, x_all[:, t, :], h[:], eps_t)
                for ch in range(8):
                    tp = pst.tile([P, P], BF16, tag="tp")
                    nc.tensor.transpose(out=tp[:], in_=h[:, ch * P:(ch + 1) * P],
                                        identity=ident[:])
                    nc.scalar.activation(out=hT[:, ch, ts], in_=tp[:], func=AF.Copy)
                lg = psa.tile([P, 48], F32, tag="sm")
                for ch in range(8):
                    nc.tensor.matmul(out=lg[:], lhsT=hT[:, ch, ts], rhs=rQKV[:, ch, :],
                                     start=(ch == 0), stop=(ch == 7))
                wQKV = sb.tile([P, 48], F32, tag="wQKV")
                for rr in range(3):
                    _softmax16(nc, sb, lg[:, rr * 16:(rr + 1) * 16],
                               wQKV[:, rr * 16:(rr + 1) * 16])
                p1a = psb.tile([P, KC], F32, tag="big")
                p1b = psb.tile([P, KC], F32, tag="big")
                for half, pt in ((0, p1a), (1, p1b)):
                    for col in range(2):
                        c0 = half * KC + col * 512
                        for ch in range(8):
                            nc.tensor.matmul(out=pt[:, col * 512:(col + 1) * 512],
                                             lhsT=hT[:, ch, ts],
                                             rhs=neur[:, ch, c0:c0 + 512],
                                             start=(ch == 0), stop=(ch == 7))
                for rr, dst in ((0, hQT), (1, hKT), (2, hVT)):
                    hc = sb.tile([P, R], BF16, tag="hc")
                    _combine(nc, sb, (p1a, p1b), wQKV[:, rr * 16:(rr + 1) * 16], hc[:])
                    tp = pst.tile([P, P], BF16, tag="tp")
                    nc.tensor.transpose(out=tp[:], in_=hc[:], identity=ident[:])
                    nc.scalar.activation(out=dst[:, ts], in_=tp[:], func=AF.Copy)

            # =========== S2: Q^T / K^T (all 16 heads) and V ===========
            for ch in range(8):
                for w_, hsrc, dst in ((wq, hQT, QT_sb), (wk, hKT, KT_sb)):
                    pr = pst.tile([P, TOK], F32, tag="tp")
                    nc.tensor.matmul(out=pr[:], lhsT=w_[:, ch * P:(ch + 1) * P],
                                     rhs=hsrc[:], start=True, stop=True)
                    nc.scalar.activation(out=dst[:, ch, :], in_=pr[:], func=AF.Copy)
            for t in range(NT):
                pv = psb.tile([P, D], F32, tag="big")
                for col in range(2):
                    nc.tensor.matmul(out=pv[:, col * 512:(col + 1) * 512],
                                     lhsT=hVT[:, t * P:(t + 1) * P],
                                     rhs=wv[:, col * 512:(col + 1) * 512],
                                     start=True, stop=True)
                nc.scalar.activation(out=V_sb[:, t, :], in_=pv[:], func=AF.Copy)

            # =========== S3: pair AllGather of QT/KT/V ===========
            groups = [[0, 1], [2, 3], [4, 5], [6, 7]]
            xin = dram.tile([P, 12288], BF16)
            xout = dram.tile([2 * P, 12288], BF16)
            nc.gpsimd.dma_start(out=xin[:, 0:4096],
                                in_=QT_sb[:].rearrange("p c t -> p (c t)"))
            nc.gpsimd.dma_start(out=xin[:, 4096:8192],
                                in_=KT_sb[:].rearrange("p c t -> p (c t)"))
            nc.gpsimd.dma_start(out=xin[:, 8192:12288],
                                in_=V_sb[:].rearrange("p c t -> p (c t)"))
            nc.gpsimd.collective_compute("AllGather", OP.bypass,
                                         replica_groups=groups,
                                         ins=[xin.opt()], outs=[xout.opt()])
            # reuse the big persistent slots for the assembled full-seq tensors
            QT_f = pp.tile([P, 4, S], BF16, tag="qt")
            KT_f = pp.tile([P, 4, S], BF16, tag="kt")
            V_f = pp.tile([P, 8, 512], BF16, tag="vv")
            for src in range(2):
                rs = slice(src * P, (src + 1) * P)
                qsl = slice(src * TOK, (src + 1) * TOK)
                for i in range(4):
                    nc.gpsimd.dma_start(
                        out=QT_f[:, i, qsl],
                        in_=xout[rs, 0:4096][:, bass.ds(off2048 + i * TOK, TOK)])
                    nc.gpsimd.dma_start(
                        out=KT_f[:, i, qsl],
                        in_=xout[rs, 4096:8192][:, bass.ds(off2048 + i * TOK, TOK)])
                    nc.gpsimd.dma_start(
                        out=V_f[:, src * 4 + i, :],
                        in_=xout[rs, 8192:12288][:, bass.ds(off512 + i * D, 512)])

            # =========== S4: causal attention, 8 heads, full sequence ===========
            attnT = pp.tile([P, 4, S], BF16, tag="at")
            for hh in range(8):
                ch, poff = hh // 2, (hh % 2) * 64
                prow = slice(poff, poff + 64)
                for qg in range(8):
                    kr = (qg + 1) * P
                    sc = psb.tile([P, S], F32, tag="big")
                    for part in range((kr + 511) // 512):
                        k0, k1 = part * 512, min(kr, (part + 1) * 512)
                        nc.tensor.matmul(out=sc[:, k0:k1],
                                         lhsT=QT_f[prow, ch, qg * P:(qg + 1) * P],
                                         rhs=KT_f[prow, ch, k0:k1],
                                         start=True, stop=True)
                    mtmp = sb.tile([P, P], F32, tag="mtmp")
                    nc.vector.tensor_tensor(out=mtmp[:], in0=sc[:, qg * P:kr],
                                            in1=tri[:], op=OP.add)
                    Pb = sb.tile([P, S], BF16, tag="Pb")
                    s2 = sb.tile([P, 1], F32, tag="s2")
                    if qg > 0:
                        s1 = sb.tile([P, 1], F32, tag="s1")
                        nc.scalar.activation(out=Pb[:, 0:qg * P], in_=sc[:, 0:qg * P],
                                             func=AF.Exp, scale=0.125, accum_out=s1[:])
                    nc.scalar.activation(out=Pb[:, qg * P:kr], in_=mtmp[:],
                                         func=AF.Exp, scale=0.125, accum_out=s2[:])
                    den = sb.tile([P, 1], F32, tag="den")
                    if qg > 0:
                        nc.vector.tensor_tensor(out=den[:], in0=s1[:], in1=s2[:],
                                                op=OP.add)
                    else:
                        nc.vector.tensor_copy(out=den[:], in_=s2[:])
                    nc.vector.reciprocal(out=den[:], in_=den[:])
                    diag = sb.tile([P, P], BF16, tag="diag")
                    nc.vector.tensor_tensor(out=diag[:], in0=ident[:],
                                            in1=den[:].to_broadcast([P, P]),
                                            op=OP.mult)
                    at = psa.tile([64, P], F32, tag="sm")
                    for kb in range(qg + 1):
                        ptp = pst.tile([P, P], F32, tag="tp")
                        nc.tensor.matmul(out=ptp[:],
                                         lhsT=Pb[:, kb * P:(kb + 1) * P],
                                         rhs=diag[:], start=True, stop=True)
                        pts = sb.tile([P, P], BF16, tag="pts")
                        nc.scalar.activation(out=pts[:], in_=ptp[:], func=AF.Copy)
                        nc.tensor.matmul(out=at[:],
                                         lhsT=V_f[:, kb, hh * 64:(hh + 1) * 64],
                                         rhs=pts[:], start=(kb == 0), stop=(kb == qg))
                    nc.scalar.activation(out=attnT[prow, ch, qg * P:(qg + 1) * P],
                                         in_=at[:], func=AF.Copy)

            # =========== S5: exchange attn^T, W_O, residual ===========
            xin2 = dram.tile([P, 4 * S], BF16)
            xout2 = dram.tile([2 * P, 4 * S], BF16)
            nc.gpsimd.dma_start(out=xin2[:], in_=attnT[:].rearrange("p c q -> p (c q)"))
            nc.gpsimd.collective_compute("AllGather", OP.bypass,
                                         replica_groups=groups,
                                         ins=[xin2.opt()], outs=[xout2.opt()])
            aT = pp.tile([P, 8, TOK], BF16, tag="at")
            for src in range(2):
                rs = slice(src * P, (src + 1) * P)
                for i in range(4):
                    nc.gpsimd.dma_start(
                        out=aT[:, src * 4 + i, :],
                        in_=xout2[rs, :][:, bass.ds(off512 + i * S, TOK)])
            for t in range(NT):
                ts = slice(t * P, (t + 1) * P)
                po = psb.tile([P, D], F32, tag="big")
                for col in range(2):
                    for ch in range(8):
                        nc.tensor.matmul(out=po[:, col * 512:(col + 1) * 512],
                                         lhsT=aT[:, ch, ts],
                                         rhs=wo[:, ch, col * 512:(col + 1) * 512],
                                         start=(ch == 0), stop=(ch == 7))
                nc.vector.tensor_tensor(out=x_all[:, t, :], in0=po[:],
                                        in1=x_all[:, t, :], op=OP.add)

            # =========== S6: LN2 + compress M -> Qm^T (into hQT) ===========
            for t in range(NT):
                ts = slice(t * P, (t + 1) * P)
                h2 = sb.tile([P, D], BF16, tag="h")
                _ln(nc, sb, x_all[:, t, :], h2[:], eps_t)
                for ch in range(8):
                    tp = pst.tile([P, P], BF16, tag="tp")
                    nc.tensor.transpose(out=tp[:], in_=h2[:, ch * P:(ch + 1) * P],
                                        identity=ident[:])
                    nc.scalar.activation(out=hT[:, ch, ts], in_=tp[:], func=AF.Copy)
                lgm = psa.tile([P, NCMP], F32, tag="sm")
                for ch in range(8):
                    nc.tensor.matmul(out=lgm[:], lhsT=hT[:, ch, ts], rhs=rM[:, ch, :],
                                     start=(ch == 0), stop=(ch == 7))
                wM = sb.tile([P, NCMP], F32, tag="wM")
                _softmax16(nc, sb, lgm[:], wM[:])
                p1a = psb.tile([P, KC], F32, tag="big")
                p1b = psb.tile([P, KC], F32, tag="big")
                for half, pt in ((0, p1a), (1, p1b)):
                    for col in range(2):
                        c0 = half * KC + col * 512
                        for ch in range(8):
                            nc.tensor.matmul(out=pt[:, col * 512:(col + 1) * 512],
                                             lhsT=hT[:, ch, ts],
                                             rhs=neur[:, ch, c0:c0 + 512],
                                             start=(ch == 0), stop=(ch == 7))
                qm = sb.tile([P, R], BF16, tag="hc")
                _combine(nc, sb, (p1a, p1b), wM[:], qm[:])
                tp = pst.tile([P, P], BF16, tag="tp")
                nc.tensor.transpose(out=tp[:], in_=qm[:], identity=ident[:])
                nc.scalar.activation(out=hQT[:, ts], in_=tp[:], func=AF.Copy)

            # =========== S7a: AllGather Qm^T across all 8 cores ===========
            xin3 = dram.tile([P, TOK], BF16)
            xout3 = dram.tile([N_CORES * P, TOK], BF16)
            nc.gpsimd.dma_start(out=xin3[:], in_=hQT[:])
            nc.gpsimd.collective_compute("AllGather", OP.bypass,
                                         replica_groups=group8,
                                         ins=[xin3.opt()], outs=[xout3.opt()])
            QmT_f = pp.tile([P, N_CORES, TOK], BF16, tag="qt")
            for c in range(N_CORES):
                nc.gpsimd.dma_start(out=QmT_f[:, c, :],
                                    in_=xout3[c * P:(c + 1) * P, :])

            # ===== S7b: scores vs local shard + local top-8, all 32 tiles =====
            cands = pp.tile([P, NTT, NKC * 8], U32, tag="cands")
            top8a = pp.tile([P, NTT, 8], U32, tag="top8a")
            for q in range(NTT):
                lq = QmT_f[:, q // 4, (q % 4) * P:(q % 4 + 1) * P]
                for ch in range(NKC):
                    ks = psb.tile([P, KC], F32, tag="big")
                    for col in range(2):
                        c0 = ch * KC + col * 512
                        nc.tensor.matmul(out=ks[:, col * 512:(col + 1) * 512],
                                         lhsT=lq, rhs=kkt[:, c0:c0 + 512],
                                         start=True, stop=True)
                    pk = packed[(q * NKC + ch) % 3]
                    nc.scalar.activation(out=pk.bitcast(U16)[:, 1::2].bitcast(BF16),
                                         in_=ks[:], func=AF.Copy)
                    c8 = cands[:, q, ch * 8:(ch + 1) * 8]
                    nc.vector.max(out=c8.bitcast(F32), in_=pk.bitcast(F32)[:])
                t8 = top8a[:, q, :]
                nc.vector.max(out=t8.bitcast(F32), in_=cands.bitcast(F32)[:, q, :])

            # =========== S7c: AllGather packed top-8 candidates ===========
            xin4 = dram.tile([P, NTT * 8], U32)
            xout4 = dram.tile([N_CORES * P, NTT * 8], U32)
            nc.gpsimd.dma_start(out=xin4[:],
                                in_=top8a[:].rearrange("p t s -> p (t s)"))
            nc.gpsimd.collective_compute("AllGather", OP.bypass,
                                         replica_groups=group8,
                                         ins=[xin4.opt()], outs=[xout4.opt()])
            cand_all = pp.tile([P, NTT, N_CORES * 8], U32, tag="hT")
            for c in range(N_CORES):
                nc.sync.dma_start(
                    out=cand_all[:, :, c * 8:(c + 1) * 8],
                    in_=xout4[c * P:(c + 1) * P, :].rearrange("p (t s) -> p t s", s=8))

            # ==== S7d: per-token softmax stats, my weights, decode my idx ====
            m8_all = pp.tile([P, NTT, 8], F32, tag="m8a")
            for q in range(NTT):
                nc.vector.max(out=m8_all[:, q, :], in_=cand_all.bitcast(F32)[:, q, :])
            # all-candidate scores, exp, threshold mask, Z
            # (scores are tiny, |s| < 1, so the usual max-subtraction before
            # exp is unnecessary; softmax is shift-invariant)
            s_all = pp.tile([P, NTT, N_CORES * 8], F32, tag="kt")
            nc.vector.tensor_scalar(out=s_all[:].bitcast(U32), in0=cand_all[:],
                                    scalar1=0xFFFF0000, scalar2=None,
                                    op0=OP.bitwise_and)
            ex_all = pp.tile([P, NTT, N_CORES * 8], F32, tag="vv")
            nc.scalar.activation(out=ex_all[:], in_=s_all[:], func=AF.Exp,
                                 scale=SCALE_R)
            mask_all = pp.tile([P, NTT, N_CORES * 8], F32, tag="mska")
            nc.vector.tensor_tensor(out=mask_all[:], in0=cand_all.bitcast(F32)[:],
                                    in1=m8_all[:, :, 7:8].to_broadcast(
                                        [P, NTT, N_CORES * 8]),
                                    op=OP.is_ge)
            nc.vector.tensor_tensor(out=ex_all[:], in0=ex_all[:], in1=mask_all[:],
                                    op=OP.mult)
            zz = pp.tile([P, NTT, 1], F32, tag="zz")
            nc.vector.tensor_reduce(out=zz[:], in_=ex_all[:], axis=AX.X, op=OP.add)
            nc.vector.reciprocal(out=zz[:], in_=zz[:])
            # my candidates: scores, exp, mask, weights
            s8a = pp.tile([P, NTT, 8], F32, tag="s8a")
            nc.vector.tensor_scalar(out=s8a[:].bitcast(U32), in0=top8a[:],
                                    scalar1=0xFFFF0000, scalar2=None,
                                    op0=OP.bitwise_and)
            w8_all = pp.tile([P, NTT, 8], F32, tag="w8a")
            nc.scalar.activation(out=w8_all[:], in_=s8a[:], func=AF.Exp,
                                 scale=SCALE_R)
            msk8 = pp.tile([P, NTT, 8], F32, tag="msk8")
            nc.vector.tensor_tensor(out=msk8[:], in0=top8a.bitcast(F32)[:],
                                    in1=m8_all[:, :, 7:8].to_broadcast([P, NTT, 8]),
                                    op=OP.is_ge)
            nc.vector.tensor_tensor(out=w8_all[:], in0=w8_all[:], in1=msk8[:],
                                    op=OP.mult)
            nc.vector.tensor_tensor(out=w8_all[:], in0=w8_all[:],
                                    in1=zz[:].to_broadcast([P, NTT, 8]),
                                    op=OP.mult)
            # decode my local knowledge-row indices
            pos_all = pp.tile([P, NTT, 8], U32, tag="posa")
            for q in range(NTT):
                nc.vector.max_index(out=pos_all[:, q, :],
                                    in_max=top8a.bitcast(F32)[:, q, :],
                                    in_values=cands.bitcast(F32)[:, q, :])
            idx_all = pp.tile([P, NTT, 8], U32, tag="idxa")
            nc.vector.tensor_scalar(out=idx_all[:], in0=pos_all[:],
                                    scalar1=3, scalar2=10,
                                    op0=OP.logical_shift_right,
                                    op1=OP.logical_shift_left)
            loc_all = pp.tile([P, NTT, 8], U32, tag="loca")
            nc.vector.tensor_scalar(out=loc_all[:], in0=top8a[:],
                                    scalar1=6, scalar2=0x3FF,
                                    op0=OP.logical_shift_right,
                                    op1=OP.bitwise_and)
            nc.vector.tensor_tensor(out=idx_all[:], in0=idx_all[:], in1=loc_all[:],
                                    op=OP.bitwise_or)

            # ==== S7e: gather my V rows, weighted partials, ReduceScatter ====
            rsin = dram.tile([NTT * P, D], BF16)
            rsout = dram.tile([NT * P, D], BF16)
            for q in range(NTT):
                acc = sb.tile([P, D], F32, tag="acc")
                for j in range(KK):
                    vg = gp.tile([P, D], BF16, tag="vg")
                    nc.gpsimd.indirect_dma_start(
                        out=vg[:], out_offset=None, in_=kV_in[:],
                        in_offset=bass.IndirectOffsetOnAxis(
                            ap=idx_all[:, q, j:j + 1], axis=0))
                    if j == 0:
                        nc.vector.tensor_scalar(out=acc[:], in0=vg[:],
                                                scalar1=w8_all[:, q, 0:1],
                                                scalar2=None, op0=OP.mult)
                    else:
                        nc.vector.scalar_tensor_tensor(out=acc[:], in0=vg[:],
                                                       scalar=w8_all[:, q, j:j + 1],
                                                       in1=acc[:], op0=OP.mult,
                                                       op1=OP.add)
                accb = sb.tile([P, D], BF16, tag="accb")
                nc.scalar.activation(out=accb[:], in_=acc[:], func=AF.Copy)
                nc.sync.dma_start(out=rsin[q * P:(q + 1) * P, :], in_=accb[:])
            nc.gpsimd.collective_compute("ReduceScatter", OP.add,
                                         replica_groups=group8,
                                         ins=[rsin.opt()], outs=[rsout.opt()])
            for t in range(NT):
                ts = slice(t * P, (t + 1) * P)
                mem = gp.tile([P, D], BF16, tag="vg")
                nc.sync.dma_start(out=mem[:], in_=rsout[t * P:(t + 1) * P, :])
                outsb = sb.tile([P, D], BF16, tag="outsb")
                nc.vector.tensor_tensor(out=outsb[:], in0=mem[:],
                                        in1=x_all[:, t, :], op=OP.add)
                nc.sync.dma_start(out=out_t[ts, :], in_=outsb[:])

    nc.finalize()
    return nc


@functools.lru_cache(maxsize=1)
def _get_program():
    return build_program()


def _prep_core_inputs(inputs):
    bf = ml_dtypes.bfloat16
    x = np.asarray(inputs["x"], np.float32).astype(bf)
    neurons = np.asarray(inputs["compress_neurons"], np.float32)
    neur_flat = np.ascontiguousarray(
        neurons.transpose(1, 0, 2).reshape(D, NCMP * R)).astype(bf)
    rt_full = np.concatenate([np.asarray(inputs["router_Q"], np.float32),
                              np.asarray(inputs["router_K"], np.float32),
                              np.asarray(inputs["router_V"], np.float32),
                              np.asarray(inputs["router_M"], np.float32)],
                             axis=1).astype(bf)
    wq = np.asarray(inputs["W_Q"], np.float32).astype(bf)
    wk = np.asarray(inputs["W_K"], np.float32).astype(bf)
    wv = np.asarray(inputs["W_V"], np.float32).astype(bf)
    wo = np.asarray(inputs["W_O"], np.float32).astype(bf)
    kKT_full = np.ascontiguousarray(
        np.asarray(inputs["knowledge_K"], np.float32).T).astype(bf)
    kV_full = np.asarray(inputs["knowledge_V"], np.float32).astype(bf)
    in_maps = []
    for c in range(N_CORES):
        b, hf = c // 2, c % 2
        rs = slice(c * (D // 8), (c + 1) * (D // 8))
        ws = slice(c * 16, (c + 1) * 16)
        m = dict(
            x_shard=np.ascontiguousarray(x[b, hf * TOK:(hf + 1) * TOK, :]),
            offs=np.array([[hf * 2048, hf * 512]], np.uint32),
            neur_sh=np.ascontiguousarray(neur_flat[rs, :]),
            rt_sh=np.ascontiguousarray(rt_full[rs, :]),
            wqkv_sh=np.ascontiguousarray(
                np.concatenate([wq[ws, :], wk[ws, :], wv[ws, :]], axis=0)),
            wo_sh=np.ascontiguousarray(wo[rs, :]),
            kKT=np.ascontiguousarray(kKT_full[:, c * NKS:(c + 1) * NKS]),
            kV=np.ascontiguousarray(kV_full[c * NKS:(c + 1) * NKS, :]),
        )
        in_maps.append(m)
    return in_maps


def kernel(**inputs) -> np.ndarray:
    nc = _get_program()
    in_maps = _prep_core_inputs(inputs)
    res = run_bass_kernel_spmd(nc, in_maps, list(range(N_CORES)))
    out = np.empty((B, S, D), np.float32)
    for c in range(N_CORES):
        b, hf = c // 2, c % 2
        out[b, hf * TOK:(hf + 1) * TOK, :] = np.asarray(
            res.results[c]["out_shard"], dtype=np.float32)
    return out


# revision 16
# speedup vs baseline: 8.0876x; 1.2489x over previous
"""DAWNBlock Trainium2 kernel (8 NeuronCores, SPMD, single NEFF launch).

Sharding: tokens split over cores as (batch b = c//2, seq-half hf = c%2),
512 tokens per core. Attention is sharded by (batch, head-group): after a
pair AllGather of Q^T/K^T/V each core runs causal attention for 8 heads over
the full 1024-token sequence of its batch; a second pair AllGather exchanges
attn^T so each core projects (W_O) only its own 512 tokens.

The knowledge stage is expert-sharded to avoid replicating the big tables:
each core holds 1/8 of knowledge_K^T ([128, 4096] bf16) and knowledge_V
([4096, 1024] bf16). Qm^T is all-gathered (tiny) so every core scores all
4096 tokens against its shard, takes a local top-8 per token with the
hardware max8 instruction over packed floats (bf16 score in the high 16
bits, in-chunk column in the low bits), and all-gathers the packed top-8
candidate lists. From the 64 gathered candidates per token every core
derives identical softmax stats (max, 8th-largest threshold, masked Z),
weights its own surviving candidates, gathers its local V rows via
indirect DMA, and a ReduceScatter sums partial outputs back to the token
owners.

Core-parity-dependent data movement (which half of the pair AllGather output
belongs to this core) is handled with register-backed dynamic DMA slices
(bass.ds) driven by a tiny per-core offsets input, so all 8 cores share one
instruction stream.
"""
import functools
import numpy as np
import ml_dtypes

import concourse.bass as bass
import concourse.bacc as bacc
import concourse.mybir as mybir
import concourse.tile as tile
from concourse.bass_utils import run_bass_kernel_spmd

F32 = mybir.dt.float32
BF16 = mybir.dt.bfloat16
F8 = mybir.dt.float8e4
U32 = mybir.dt.uint32
U16 = mybir.dt.uint16
AF = mybir.ActivationFunctionType
OP = mybir.AluOpType
AX = mybir.AxisListType

N_CORES = 8
P = 128
D = 1024
R = 128
NCMP = 16
NK = 32768
NKS = NK // N_CORES    # 4096 knowledge rows per core
KK = 8
S = 1024
B = 4
TOK = 512
NT = TOK // P          # 4 token tiles per core
NTT = B * S // P       # 32 token tiles globally
EPS = 1e-5
NEG = -1.0e30
KC = 1024              # knowledge-score chunk width
NKC = NKS // KC        # 4 chunks per core shard
SCALE_R = float(1.0 / np.sqrt(R))


def _ln(nc, sb, x_ap, out_ap, eps_tile):
    """LayerNorm (gamma=1, beta=0): x_ap [128, D] f32 -> out_ap (bf16)."""
    stats = sb.tile([P, 2, 6], F32, tag="ln_stats")
    for g in range(2):
        nc.vector.bn_stats(out=stats[:, g, :], in_=x_ap[:, g * 512:(g + 1) * 512])
    mv = sb.tile([P, 2], F32, tag="ln_mv")
    nc.vector.bn_aggr(out=mv[:], in_=stats[:])
    rstd = sb.tile([P, 1], F32, tag="ln_rstd")
    nc.scalar.activation(out=rstd[:], in_=mv[:, 1:2], func=AF.Sqrt,
                         bias=eps_tile[:], scale=1.0)
    nc.vector.reciprocal(out=rstd[:], in_=rstd[:])
    nc.vector.tensor_scalar(out=out_ap, in0=x_ap, scalar1=mv[:, 0:1],
                            scalar2=rstd[:], op0=OP.subtract, op1=OP.mult)


def _softmax16(nc, sb, logits_ap, w_ap):
    """softmax over 16 router logits (PSUM f32 in) -> w_ap [128,16] f32."""
    mx = sb.tile([P, 1], F32, tag="rs_mx")
    nc.vector.tensor_reduce(out=mx[:], in_=logits_ap, axis=AX.X, op=OP.max)
    nmx = sb.tile([P, 1], F32, tag="rs_nmx")
    nc.vector.tensor_scalar_mul(out=nmx[:], in0=mx[:], scalar1=-1.0)
    ssum = sb.tile([P, 1], F32, tag="rs_sum")
    nc.scalar.activation(out=w_ap, in_=logits_ap, func=AF.Exp,
                         bias=nmx[:], scale=1.0, accum_out=ssum[:])
    nc.vector.reciprocal(out=ssum[:], in_=ssum[:])
    nc.vector.tensor_scalar_mul(out=w_ap, in0=w_ap, scalar1=ssum[:])


def _combine(nc, sb, p1_halves, w_ap, out_ap):
    """out[t,:] = sum_n w[t,n] * P1[t, n*128:(n+1)*128] (P1 in 2 PSUM halves)."""
    acc = sb.tile([P, R], F32, tag="cmb_acc")
    for n in range(NCMP):
        src = p1_halves[n // 8][:, (n % 8) * R:(n % 8 + 1) * R]
        if n == 0:
            nc.vector.tensor_scalar(out=acc[:], in0=src, scalar1=w_ap[:, 0:1],
                                    scalar2=None, op0=OP.mult)
        else:
            nc.vector.scalar_tensor_tensor(out=acc[:], in0=src,
                                           scalar=w_ap[:, n:n + 1], in1=acc[:],
                                           op0=OP.mult, op1=OP.add)
    nc.vector.tensor_copy(out=out_ap, in_=acc[:])


def build_program():
    nc = bacc.Bacc(None, num_devices=N_CORES)

    x_in = nc.dram_tensor("x_shard", [TOK, D], BF16, kind="ExternalInput")
    neur_in = nc.dram_tensor("neur_sh", [D // 8, NCMP * R], BF16, kind="ExternalInput")
    rt_in = nc.dram_tensor("rt_sh", [D // 8, 64], BF16, kind="ExternalInput")
    wqkv_in = nc.dram_tensor("wqkv_sh", [48, D], BF16, kind="ExternalInput")
    wo_in = nc.dram_tensor("wo_sh", [D // 8, D], BF16, kind="ExternalInput")
    kKT_in = nc.dram_tensor("kKT", [R, NKS], BF16, kind="ExternalInput")
    kV_in = nc.dram_tensor("kV", [NKS, D], F8, kind="ExternalInput")
    offs_in = nc.dram_tensor("offs", [1, 2], U32, kind="ExternalInput")
    out_t = nc.dram_tensor("out_shard", [TOK, D], BF16, kind="ExternalOutput")

    with tile.TileContext(nc) as tc:
        with (
            tc.tile_pool(name="persist", bufs=1) as pp,
            tc.tile_pool(name="weights", bufs=1) as wp,
            tc.tile_pool(name="work", bufs=2) as sb,
            tc.tile_pool(name="gath", bufs=3) as gp,
            tc.tile_pool(name="ps_big", bufs=2, space="PSUM") as psb,
            tc.tile_pool(name="ps_tp", bufs=2, space="PSUM") as pst,
            tc.tile_pool(name="ps_sm", bufs=2, space="PSUM") as psa,
            tc.tile_pool(name="dram", bufs=1, space="DRAM") as dram,
        ):
            # ---- parity offsets -> gpsimd registers for dynamic DMA slices ----
            r2048 = nc.gpsimd.alloc_register("off2048")
            nc.gpsimd.reg_load(r2048, offs_in[0:1, 0:1])
            off2048 = nc.gpsimd.snap(r2048, donate=True, min_val=0, max_val=2048)
            r512 = nc.gpsimd.alloc_register("off512")
            nc.gpsimd.reg_load(r512, offs_in[0:1, 1:2])
            off512 = nc.gpsimd.snap(r512, donate=True, min_val=0, max_val=512)

            group8 = [list(range(N_CORES))]

            # ---- broadcast replicated weights on-device (1/8 shard shipped
            # from host per core; AllGather along rows reassembles the full
            # tensor in rank==row-block order) ----
            stgN = dram.tile([D // 8, NCMP * R], BF16)
            nc.gpsimd.dma_start(out=stgN[:], in_=neur_in[:])
            agN = dram.tile([D, NCMP * R], BF16)
            nc.gpsimd.collective_compute("AllGather", OP.bypass,
                                         replica_groups=group8,
                                         ins=[stgN.opt()], outs=[agN.opt()])
            stgR = dram.tile([D // 8, 64], BF16)
            nc.gpsimd.dma_start(out=stgR[:], in_=rt_in[:])
            agR = dram.tile([D, 64], BF16)
            nc.gpsimd.collective_compute("AllGather", OP.bypass,
                                         replica_groups=group8,
                                         ins=[stgR.opt()], outs=[agR.opt()])
            stgW = dram.tile([48, D], BF16)
            nc.gpsimd.dma_start(out=stgW[:], in_=wqkv_in[:])
            agW = dram.tile([384, D], BF16)
            nc.gpsimd.collective_compute("AllGather", OP.bypass,
                                         replica_groups=group8,
                                         ins=[stgW.opt()], outs=[agW.opt()])
            stgO = dram.tile([D // 8, D], BF16)
            nc.gpsimd.dma_start(out=stgO[:], in_=wo_in[:])
            agO = dram.tile([D, D], BF16)
            nc.gpsimd.collective_compute("AllGather", OP.bypass,
                                         replica_groups=group8,
                                         ins=[stgO.opt()], outs=[agO.opt()])

            # ---- resident weights ----
            neur = wp.tile([P, 8, NCMP * R], BF16)
            nc.sync.dma_start(out=neur[:], in_=agN[:].rearrange("(c p) n -> p c n", p=P))
            rQKV = wp.tile([P, 8, 48], BF16)
            nc.sync.dma_start(out=rQKV[:],
                              in_=agR[:, 0:48].rearrange("(c p) n -> p c n", p=P))
            rM = wp.tile([P, 8, NCMP], BF16)
            nc.sync.dma_start(out=rM[:],
                              in_=agR[:, 48:64].rearrange("(c p) n -> p c n", p=P))
            wq = wp.tile([P, D], BF16)
            wk = wp.tile([P, D], BF16)
            wv = wp.tile([P, D], BF16)
            for c in range(N_CORES):
                for w_i, w_t in enumerate((wq, wk, wv)):
                    nc.sync.dma_start(
                        out=w_t[c * 16:(c + 1) * 16, :],
                        in_=agW[c * 48 + w_i * 16:c * 48 + (w_i + 1) * 16, :])
            wo = wp.tile([P, 8, D], BF16)
            nc.sync.dma_start(out=wo[:], in_=agO[:].rearrange("(c p) n -> p c n", p=P))
            kkt = wp.tile([P, NKS], BF16)
            nc.sync.dma_start(out=kkt[:], in_=kKT_in[:])
            eps_t = wp.tile([P, 1], F32)
            nc.vector.memset(eps_t[:], EPS)
            iota_t = wp.tile([P, KC], U16)
            nc.gpsimd.iota(out=iota_t[:], pattern=[[64, KC]], base=0,
                           channel_multiplier=0)

            # ---- generate ident (bf16 I) and tri (0 / -1e30 causal) ----
            colx = wp.tile([P, P], F32)
            nc.gpsimd.iota(out=colx[:], pattern=[[1, P]], base=0,
                           channel_multiplier=0,
                           allow_small_or_imprecise_dtypes=True)
            rowx = wp.tile([P, 1], F32)
            nc.gpsimd.iota(out=rowx[:], pattern=[[0, 1]], base=0,
                           channel_multiplier=1,
                           allow_small_or_imprecise_dtypes=True)
            ident = wp.tile([P, P], BF16)
            nc.vector.tensor_scalar(out=ident[:], in0=colx[:], scalar1=rowx[:],
                                    scalar2=None, op0=OP.is_equal)
            tri = wp.tile([P, P], F32)
            nc.vector.tensor_scalar(out=tri[:], in0=colx[:], scalar1=rowx[:],
                                    scalar2=NEG, op0=OP.is_gt, op1=OP.mult)

            # ---- persistent activations ----
            x_all = pp.tile([P, NT, D], F32)
            hT = pp.tile([P, 8, TOK], BF16, tag="hT")
            hQT = pp.tile([P, TOK], BF16, tag="hQT")
            hKT = pp.tile([P, TOK], BF16, tag="hKT")
            hVT = pp.tile([P, TOK], BF16, tag="hVT")
            QT_sb = pp.tile([P, 8, TOK], BF16, tag="qt")
            KT_sb = pp.tile([P, 8, TOK], BF16, tag="kt")
            V_sb = pp.tile([P, NT, D], BF16, tag="vv")

            # packed score buffers (iota pre-written into the low u16 lanes)
            packed = [pp.tile([P, KC], U32, tag=f"pk{i}", name=f"pk{i}") for i in range(3)]
            for pk in packed:
                nc.vector.tensor_copy(out=pk.bitcast(U16)[:, 0::2], in_=iota_t[:])

            # =========== S1: LN1, shared projection, routed compress ===========
            for t in range(NT):
                ts = slice(t * P, (t + 1) * P)
                xb = sb.tile([P, D], BF16, tag="xb")
                nc.sync.dma_start(out=xb[:], in_=x_in[ts, :])
                nc.vector.tensor_copy(out=x_all[:, t, :], in_=xb[:])
                h = sb.tile([P, D], BF16, tag="h")
                _ln(nc, sb, x_all[:, t, :], h[:], eps_t)
                for ch in range(8):
                    tp = pst.tile([P, P], BF16, tag="tp")
                    nc.tensor.transpose(out=tp[:], in_=h[:, ch * P:(ch + 1) * P],
                                        identity=ident[:])
                    nc.scalar.activation(out=hT[:, ch, ts], in_=tp[:], func=AF.Copy)
                lg = psa.tile([P, 48], F32, tag="sm")
                for ch in range(8):
                    nc.tensor.matmul(out=lg[:], lhsT=hT[:, ch, ts], rhs=rQKV[:, ch, :],
                                     start=(ch == 0), stop=(ch == 7))
                wQKV = sb.tile([P, 48], F32, tag="wQKV")
                for rr in range(3):
                    _softmax16(nc, sb, lg[:, rr * 16:(rr + 1) * 16],
                               wQKV[:, rr * 16:(rr + 1) * 16])
                p1a = psb.tile([P, KC], F32, tag="big")
                p1b = psb.tile([P, KC], F32, tag="big")
                for half, pt in ((0, p1a), (1, p1b)):
                    for col in range(2):
                        c0 = half * KC + col * 512
                        for ch in range(8):
                            nc.tensor.matmul(out=pt[:, col * 512:(col + 1) * 512],
                                             lhsT=hT[:, ch, ts],
                                             rhs=neur[:, ch, c0:c0 + 512],
                                             start=(ch == 0), stop=(ch == 7))
                for rr, dst in ((0, hQT), (1, hKT), (2, hVT)):
                    hc = sb.tile([P, R], BF16, tag="hc")
                    _combine(nc, sb, (p1a, p1b), wQKV[:, rr * 16:(rr + 1) * 16], hc[:])
                    tp = pst.tile([P, P], BF16, tag="tp")
                    nc.tensor.transpose(out=tp[:], in_=hc[:], identity=ident[:])
                    nc.scalar.activation(out=dst[:, ts], in_=tp[:], func=AF.Copy)

            # =========== S2: Q^T / K^T (all 16 heads) and V ===========
            for ch in range(8):
                for w_, hsrc, dst in ((wq, hQT, QT_sb), (wk, hKT, KT_sb)):
                    pr = pst.tile([P, TOK], F32, tag="tp")
                    nc.tensor.matmul(out=pr[:], lhsT=w_[:, ch * P:(ch + 1) * P],
                                     rhs=hsrc[:], start=True, stop=True)
                    nc.scalar.activation(out=dst[:, ch, :], in_=pr[:], func=AF.Copy)
            for t in range(NT):
                pv = psb.tile([P, D], F32, tag="big")
                for col in range(2):
                    nc.tensor.matmul(out=pv[:, col * 512:(col + 1) * 512],
                                     lhsT=hVT[:, t * P:(t + 1) * P],
                                     rhs=wv[:, col * 512:(col + 1) * 512],
                                     start=True, stop=True)
                nc.scalar.activation(out=V_sb[:, t, :], in_=pv[:], func=AF.Copy)

            # =========== S3: pair AllGather of QT/KT/V ===========
            groups = [[0, 1], [2, 3], [4, 5], [6, 7]]
            xin = dram.tile([P, 12288], BF16)
            xout = dram.tile([2 * P, 12288], BF16)
            nc.gpsimd.dma_start(out=xin[:, 0:4096],
                                in_=QT_sb[:].rearrange("p c t -> p (c t)"))
            nc.gpsimd.dma_start(out=xin[:, 4096:8192],
                                in_=KT_sb[:].rearrange("p c t -> p (c t)"))
            nc.gpsimd.dma_start(out=xin[:, 8192:12288],
                                in_=V_sb[:].rearrange("p c t -> p (c t)"))
            nc.gpsimd.collective_compute("AllGather", OP.bypass,
                                         replica_groups=groups,
                                         ins=[xin.opt()], outs=[xout.opt()])
            # reuse the big persistent slots for the assembled full-seq tensors
            QT_f = pp.tile([P, 4, S], BF16, tag="qt")
            KT_f = pp.tile([P, 4, S], BF16, tag="kt")
            V_f = pp.tile([P, 8, 512], BF16, tag="vv")
            for src in range(2):
                rs = slice(src * P, (src + 1) * P)
                qsl = slice(src * TOK, (src + 1) * TOK)
                for i in range(4):
                    nc.gpsimd.dma_start(
                        out=QT_f[:, i, qsl],
                        in_=xout[rs, 0:4096][:, bass.ds(off2048 + i * TOK, TOK)])
                    nc.gpsimd.dma_start(
                        out=KT_f[:, i, qsl],
                        in_=xout[rs, 4096:8192][:, bass.ds(off2048 + i * TOK, TOK)])
                    nc.gpsimd.dma_start(
                        out=V_f[:, src * 4 + i, :],
                        in_=xout[rs, 8192:12288][:, bass.ds(off512 + i * D, 512)])

            # =========== S4: causal attention, 8 heads, full sequence ===========
            attnT = pp.tile([P, 4, S], BF16, tag="at")
            for hh in range(8):
                ch, poff = hh // 2, (hh % 2) * 64
                prow = slice(poff, poff + 64)
                for qg in range(8):
                    kr = (qg + 1) * P
                    sc = psb.tile([P, S], F32, tag="big")
                    for part in range((kr + 511) // 512):
                        k0, k1 = part * 512, min(kr, (part + 1) * 512)
                        nc.tensor.matmul(out=sc[:, k0:k1],
                                         lhsT=QT_f[prow, ch, qg * P:(qg + 1) * P],
                                         rhs=KT_f[prow, ch, k0:k1],
                                         start=True, stop=True)
                    mtmp = sb.tile([P, P], F32, tag="mtmp")
                    nc.vector.tensor_tensor(out=mtmp[:], in0=sc[:, qg * P:kr],
                                            in1=tri[:], op=OP.add)
                    Pb = sb.tile([P, S], BF16, tag="Pb")
                    s2 = sb.tile([P, 1], F32, tag="s2")
                    if qg > 0:
                        s1 = sb.tile([P, 1], F32, tag="s1")
                        nc.scalar.activation(out=Pb[:, 0:qg * P], in_=sc[:, 0:qg * P],
                                             func=AF.Exp, scale=0.125, accum_out=s1[:])
                    nc.scalar.activation(out=Pb[:, qg * P:kr], in_=mtmp[:],
                                         func=AF.Exp, scale=0.125, accum_out=s2[:])
                    den = sb.tile([P, 1], F32, tag="den")
                    if qg > 0:
                        nc.vector.tensor_tensor(out=den[:], in0=s1[:], in1=s2[:],
                                                op=OP.add)
                    else:
                        nc.vector.tensor_copy(out=den[:], in_=s2[:])
                    nc.vector.reciprocal(out=den[:], in_=den[:])
                    diag = sb.tile([P, P], BF16, tag="diag")
                    nc.vector.tensor_tensor(out=diag[:], in0=ident[:],
                                            in1=den[:].to_broadcast([P, P]),
                                            op=OP.mult)
                    at = psa.tile([64, P], F32, tag="sm")
                    for kb in range(qg + 1):
                        ptp = pst.tile([P, P], F32, tag="tp")
                        nc.tensor.matmul(out=ptp[:],
                                         lhsT=Pb[:, kb * P:(kb + 1) * P],
                                         rhs=diag[:], start=True, stop=True)
                        pts = sb.tile([P, P], BF16, tag="pts")
                        nc.scalar.activation(out=pts[:], in_=ptp[:], func=AF.Copy)
                        nc.tensor.matmul(out=at[:],
                                         lhsT=V_f[:, kb, hh * 64:(hh + 1) * 64],
                                         rhs=pts[:], start=(kb == 0), stop=(kb == qg))
                    nc.scalar.activation(out=attnT[prow, ch, qg * P:(qg + 1) * P],
                                         in_=at[:], func=AF.Copy)

            # =========== S5: exchange attn^T, W_O, residual ===========
            xin2 = dram.tile([P, 4 * S], BF16)
            xout2 = dram.tile([2 * P, 4 * S], BF16)
            nc.gpsimd.dma_start(out=xin2[:], in_=attnT[:].rearrange("p c q -> p (c q)"))
            nc.gpsimd.collective_compute("AllGather", OP.bypass,
                                         replica_groups=groups,
                                         ins=[xin2.opt()], outs=[xout2.opt()])
            aT = pp.tile([P, 8, TOK], BF16, tag="at")
            for src in range(2):
                rs = slice(src * P, (src + 1) * P)
                for i in range(4):
                    nc.gpsimd.dma_start(
                        out=aT[:, src * 4 + i, :],
                        in_=xout2[rs, :][:, bass.ds(off512 + i * S, TOK)])
            for t in range(NT):
                ts = slice(t * P, (t + 1) * P)
                po = psb.tile([P, D], F32, tag="big")
                for col in range(2):
                    for ch in range(8):
                        nc.tensor.matmul(out=po[:, col * 512:(col + 1) * 512],
                                         lhsT=aT[:, ch, ts],
                                         rhs=wo[:, ch, col * 512:(col + 1) * 512],
                                         start=(ch == 0), stop=(ch == 7))
                nc.vector.tensor_tensor(out=x_all[:, t, :], in0=po[:],
                                        in1=x_all[:, t, :], op=OP.add)

            # =========== S6: LN2 + compress M -> Qm^T (into hQT) ===========
            for t in range(NT):
                ts = slice(t * P, (t + 1) * P)
                h2 = sb.tile([P, D], BF16, tag="h")
                _ln(nc, sb, x_all[:, t, :], h2[:], eps_t)
                for ch in range(8):
                    tp = pst.tile([P, P], BF16, tag="tp")
                    nc.tensor.transpose(out=tp[:], in_=h2[:, ch * P:(ch + 1) * P],
                                        identity=ident[:])
                    nc.scalar.activation(out=hT[:, ch, ts], in_=tp[:], func=AF.Copy)
                lgm = psa.tile([P, NCMP], F32, tag="sm")
                for ch in range(8):
                    nc.tensor.matmul(out=lgm[:], lhsT=hT[:, ch, ts], rhs=rM[:, ch, :],
                                     start=(ch == 0), stop=(ch == 7))
                wM = sb.tile([P, NCMP], F32, tag="wM")
                _softmax16(nc, sb, lgm[:], wM[:])
                p1a = psb.tile([P, KC], F32, tag="big")
                p1b = psb.tile([P, KC], F32, tag="big")
                for half, pt in ((0, p1a), (1, p1b)):
                    for col in range(2):
                        c0 = half * KC + col * 512
                        for ch in range(8):
                            nc.tensor.matmul(out=pt[:, col * 512:(col + 1) * 512],
                                             lhsT=hT[:, ch, ts],
                                             rhs=neur[:, ch, c0:c0 + 512],
                                             start=(ch == 0), stop=(ch == 7))
                qm = sb.tile([P, R], BF16, tag="hc")
                _combine(nc, sb, (p1a, p1b), wM[:], qm[:])
                tp = pst.tile([P, P], BF16, tag="tp")
                nc.tensor.transpose(out=tp[:], in_=qm[:], identity=ident[:])
                nc.scalar.activation(out=hQT[:, ts], in_=tp[:], func=AF.Copy)

            # =========== S7a: AllGather Qm^T across all 8 cores ===========
            xin3 = dram.tile([P, TOK], BF16)
            xout3 = dram.tile([N_CORES * P, TOK], BF16)
            nc.gpsimd.dma_start(out=xin3[:], in_=hQT[:])
            nc.gpsimd.collective_compute("AllGather", OP.bypass,
                                         replica_groups=group8,
                                         ins=[xin3.opt()], outs=[xout3.opt()])
            QmT_f = pp.tile([P, N_CORES, TOK], BF16, tag="qt")
            for c in range(N_CORES):
                nc.gpsimd.dma_start(out=QmT_f[:, c, :],
                                    in_=xout3[c * P:(c + 1) * P, :])

            # ===== S7b: scores vs local shard + local top-8, all 32 tiles =====
            cands = pp.tile([P, NTT, NKC * 8], U32, tag="cands")
            top8a = pp.tile([P, NTT, 8], U32, tag="top8a")
            for q in range(NTT):
                lq = QmT_f[:, q // 4, (q % 4) * P:(q % 4 + 1) * P]
                for ch in range(NKC):
                    ks = psb.tile([P, KC], F32, tag="big")
                    for col in range(2):
                        c0 = ch * KC + col * 512
                        nc.tensor.matmul(out=ks[:, col * 512:(col + 1) * 512],
                                         lhsT=lq, rhs=kkt[:, c0:c0 + 512],
                                         start=True, stop=True)
                    pk = packed[(q * NKC + ch) % 3]
                    nc.scalar.activation(out=pk.bitcast(U16)[:, 1::2].bitcast(BF16),
                                         in_=ks[:], func=AF.Copy)
                    c8 = cands[:, q, ch * 8:(ch + 1) * 8]
                    nc.vector.max(out=c8.bitcast(F32), in_=pk.bitcast(F32)[:])
                t8 = top8a[:, q, :]
                nc.vector.max(out=t8.bitcast(F32), in_=cands.bitcast(F32)[:, q, :])

            # =========== S7c: AllGather packed top-8 candidates ===========
            xin4 = dram.tile([P, NTT * 8], U32)
            xout4 = dram.tile([N_CORES * P, NTT * 8], U32)
            nc.gpsimd.dma_start(out=xin4[:],
                                in_=top8a[:].rearrange("p t s -> p (t s)"))
            nc.gpsimd.collective_compute("AllGather", OP.bypass,
                                         replica_groups=group8,
                                         ins=[xin4.opt()], outs=[xout4.opt()])
            cand_all = pp.tile([P, NTT, N_CORES * 8], U32, tag="hT")
            for c in range(N_CORES):
                nc.sync.dma_start(
                    out=cand_all[:, :, c * 8:(c + 1) * 8],
                    in_=xout4[c * P:(c + 1) * P, :].rearrange("p (t s) -> p t s", s=8))

            # ==== S7d: per-token softmax stats, my weights, decode my idx ====
            m8_all = pp.tile([P, NTT, 8], F32, tag="m8a")
            for q in range(NTT):
                nc.vector.max(out=m8_all[:, q, :], in_=cand_all.bitcast(F32)[:, q, :])
            # all-candidate scores, exp, threshold mask, Z
            # (scores are tiny, |s| < 1, so the usual max-subtraction before
            # exp is unnecessary; softmax is shift-invariant)
            s_all = pp.tile([P, NTT, N_CORES * 8], F32, tag="kt")
            nc.vector.tensor_scalar(out=s_all[:].bitcast(U32), in0=cand_all[:],
                                    scalar1=0xFFFF0000, scalar2=None,
                                    op0=OP.bitwise_and)
            ex_all = pp.tile([P, NTT, N_CORES * 8], F32, tag="vv")
            nc.scalar.activation(out=ex_all[:], in_=s_all[:], func=AF.Exp,
                                 scale=SCALE_R)
            mask_all = pp.tile([P, NTT, N_CORES * 8], F32, tag="mska")
            nc.vector.tensor_tensor(out=mask_all[:], in0=cand_all.bitcast(F32)[:],
                                    in1=m8_all[:, :, 7:8].to_broadcast(
                                        [P, NTT, N_CORES * 8]),
                                    op=OP.is_ge)
            nc.vector.tensor_tensor(out=ex_all[:], in0=ex_all[:], in1=mask_all[:],
                                    op=OP.mult)
            zz = pp.tile([P, NTT, 1], F32, tag="zz")
            nc.vector.tensor_reduce(out=zz[:], in_=ex_all[:], axis=AX.X, op=OP.add)
            nc.vector.reciprocal(out=zz[:], in_=zz[:])
            # fold the 1/64 fp8 kV pre-scale into the softmax normalizer
            nc.vector.tensor_scalar_mul(out=zz[:], in0=zz[:], scalar1=1.0 / 64.0)
            # my candidates: scores, exp, mask, weights
            s8a = pp.tile([P, NTT, 8], F32, tag="s8a")
            nc.vector.tensor_scalar(out=s8a[:].bitcast(U32), in0=top8a[:],
                                    scalar1=0xFFFF0000, scalar2=None,
                                    op0=OP.bitwise_and)
            w8_all = pp.tile([P, NTT, 8], F32, tag="w8a")
            nc.scalar.activation(out=w8_all[:], in_=s8a[:], func=AF.Exp,
                                 scale=SCALE_R)
            msk8 = pp.tile([P, NTT, 8], F32, tag="msk8")
            nc.vector.tensor_tensor(out=msk8[:], in0=top8a.bitcast(F32)[:],
                                    in1=m8_all[:, :, 7:8].to_broadcast([P, NTT, 8]),
                                    op=OP.is_ge)
            nc.vector.tensor_tensor(out=w8_all[:], in0=w8_all[:], in1=msk8[:],
                                    op=OP.mult)
            nc.vector.tensor_tensor(out=w8_all[:], in0=w8_all[:],
                                    in1=zz[:].to_broadcast([P, NTT, 8]),
                                    op=OP.mult)
            # decode my local knowledge-row indices
            pos_all = pp.tile([P, NTT, 8], U32, tag="posa")
            for q in range(NTT):
                nc.vector.max_index(out=pos_all[:, q, :],
                                    in_max=top8a.bitcast(F32)[:, q, :],
                                    in_values=cands.bitcast(F32)[:, q, :])
            idx_all = pp.tile([P, NTT, 8], U32, tag="idxa")
            nc.vector.tensor_scalar(out=idx_all[:], in0=pos_all[:],
                                    scalar1=3, scalar2=10,
                                    op0=OP.logical_shift_right,
                                    op1=OP.logical_shift_left)
            loc_all = pp.tile([P, NTT, 8], U32, tag="loca")
            nc.vector.tensor_scalar(out=loc_all[:], in0=top8a[:],
                                    scalar1=6, scalar2=0x3FF,
                                    op0=OP.logical_shift_right,
                                    op1=OP.bitwise_and)
            nc.vector.tensor_tensor(out=idx_all[:], in0=idx_all[:], in1=loc_all[:],
                                    op=OP.bitwise_or)

            # ==== S7e: gather my V rows, weighted partials, ReduceScatter ====
            rsin = dram.tile([NTT * P, D], BF16)
            rsout = dram.tile([NT * P, D], BF16)
            for q in range(NTT):
                acc = sb.tile([P, D], F32, tag="acc")
                for j in range(KK):
                    vg = gp.tile([P, D], F8, tag="vg8")
                    nc.gpsimd.indirect_dma_start(
                        out=vg[:], out_offset=None, in_=kV_in[:],
                        in_offset=bass.IndirectOffsetOnAxis(
                            ap=idx_all[:, q, j:j + 1], axis=0))
                    if j == 0:
                        nc.vector.tensor_scalar(out=acc[:], in0=vg[:],
                                                scalar1=w8_all[:, q, 0:1],
                                                scalar2=None, op0=OP.mult)
                    else:
                        nc.vector.scalar_tensor_tensor(out=acc[:], in0=vg[:],
                                                       scalar=w8_all[:, q, j:j + 1],
                                                       in1=acc[:], op0=OP.mult,
                                                       op1=OP.add)
                accb = sb.tile([P, D], BF16, tag="accb")
                nc.scalar.activation(out=accb[:], in_=acc[:], func=AF.Copy)
                nc.sync.dma_start(out=rsin[q * P:(q + 1) * P, :], in_=accb[:])
            nc.gpsimd.collective_compute("ReduceScatter", OP.add,
                                         replica_groups=group8,
                                         ins=[rsin.opt()], outs=[rsout.opt()])
            for t in range(NT):
                ts = slice(t * P, (t + 1) * P)
                mem = gp.tile([P, D], BF16, tag="vg")
                nc.sync.dma_start(out=mem[:], in_=rsout[t * P:(t + 1) * P, :])
                outsb = sb.tile([P, D], BF16, tag="outsb")
                nc.vector.tensor_tensor(out=outsb[:], in0=mem[:],
                                        in1=x_all[:, t, :], op=OP.add)
                nc.sync.dma_start(out=out_t[ts, :], in_=outsb[:])

    nc.finalize()
    return nc


@functools.lru_cache(maxsize=1)
def _get_program():
    return build_program()


def _prep_core_inputs(inputs):
    bf = ml_dtypes.bfloat16
    x = np.asarray(inputs["x"], np.float32).astype(bf)
    neurons = np.asarray(inputs["compress_neurons"], np.float32)
    neur_flat = np.ascontiguousarray(
        neurons.transpose(1, 0, 2).reshape(D, NCMP * R)).astype(bf)
    rt_full = np.concatenate([np.asarray(inputs["router_Q"], np.float32),
                              np.asarray(inputs["router_K"], np.float32),
                              np.asarray(inputs["router_V"], np.float32),
                              np.asarray(inputs["router_M"], np.float32)],
                             axis=1).astype(bf)
    wq = np.asarray(inputs["W_Q"], np.float32).astype(bf)
    wk = np.asarray(inputs["W_K"], np.float32).astype(bf)
    wv = np.asarray(inputs["W_V"], np.float32).astype(bf)
    wo = np.asarray(inputs["W_O"], np.float32).astype(bf)
    kKT_full = np.ascontiguousarray(
        np.asarray(inputs["knowledge_K"], np.float32).T).astype(bf)
    kV8_full = (np.asarray(inputs["knowledge_V"], np.float32) * 64.0).astype(
        ml_dtypes.float8_e4m3)
    in_maps = []
    for c in range(N_CORES):
        b, hf = c // 2, c % 2
        rs = slice(c * (D // 8), (c + 1) * (D // 8))
        ws = slice(c * 16, (c + 1) * 16)
        m = dict(
            x_shard=np.ascontiguousarray(x[b, hf * TOK:(hf + 1) * TOK, :]),
            offs=np.array([[hf * 2048, hf * 512]], np.uint32),
            neur_sh=np.ascontiguousarray(neur_flat[rs, :]),
            rt_sh=np.ascontiguousarray(rt_full[rs, :]),
            wqkv_sh=np.ascontiguousarray(
                np.concatenate([wq[ws, :], wk[ws, :], wv[ws, :]], axis=0)),
            wo_sh=np.ascontiguousarray(wo[rs, :]),
            kKT=np.ascontiguousarray(kKT_full[:, c * NKS:(c + 1) * NKS]),
            kV=np.ascontiguousarray(kV8_full[c * NKS:(c + 1) * NKS, :]),
        )
        in_maps.append(m)
    return in_maps


def kernel(**inputs) -> np.ndarray:
    nc = _get_program()
    in_maps = _prep_core_inputs(inputs)
    res = run_bass_kernel_spmd(nc, in_maps, list(range(N_CORES)))
    out = np.empty((B, S, D), np.float32)
    for c in range(N_CORES):
        b, hf = c // 2, c % 2
        out[b, hf * TOK:(hf + 1) * TOK, :] = np.asarray(
            res.results[c]["out_shard"], dtype=np.float32)
    return out


# revision 19
# speedup vs baseline: 8.3335x; 1.0304x over previous
"""DAWNBlock Trainium2 kernel (8 NeuronCores, SPMD, single NEFF launch).

Sharding: tokens split over cores as (batch b = c//2, seq-half hf = c%2),
512 tokens per core. Attention is sharded by (batch, head-group): after a
pair AllGather of Q^T/K^T/V each core runs causal attention for 8 heads over
the full 1024-token sequence of its batch; a second pair AllGather exchanges
attn^T so each core projects (W_O) only its own 512 tokens.

The knowledge stage is expert-sharded to avoid replicating the big tables:
each core holds 1/8 of knowledge_K^T ([128, 4096] bf16) and knowledge_V
([4096, 1024] bf16). Qm^T is all-gathered (tiny) so every core scores all
4096 tokens against its shard, takes a local top-8 per token with the
hardware max8 instruction over packed floats (bf16 score in the high 16
bits, in-chunk column in the low bits), and all-gathers the packed top-8
candidate lists. From the 64 gathered candidates per token every core
derives identical softmax stats (max, 8th-largest threshold, masked Z),
weights its own surviving candidates, gathers its local V rows via
indirect DMA, and a ReduceScatter sums partial outputs back to the token
owners.

Core-parity-dependent data movement (which half of the pair AllGather output
belongs to this core) is handled with register-backed dynamic DMA slices
(bass.ds) driven by a tiny per-core offsets input, so all 8 cores share one
instruction stream.
"""
import functools
import numpy as np
import ml_dtypes

import concourse.bass as bass
import concourse.bacc as bacc
import concourse.mybir as mybir
import concourse.tile as tile
from concourse.bass_utils import run_bass_kernel_spmd

F32 = mybir.dt.float32
BF16 = mybir.dt.bfloat16
F8 = mybir.dt.float8e4
U32 = mybir.dt.uint32
U16 = mybir.dt.uint16
AF = mybir.ActivationFunctionType
OP = mybir.AluOpType
AX = mybir.AxisListType

N_CORES = 8
P = 128
D = 1024
R = 128
NCMP = 16
NK = 32768
NKS = NK // N_CORES    # 4096 knowledge rows per core
KK = 8
S = 1024
B = 4
TOK = 512
NT = TOK // P          # 4 token tiles per core
NTT = B * S // P       # 32 token tiles globally
EPS = 1e-5
NEG = -1.0e30
KC = 1024              # knowledge-score chunk width
NKC = NKS // KC        # 4 chunks per core shard
SCALE_R = float(1.0 / np.sqrt(R))
SCALE_K = SCALE_R / 64.0       # knowledge_K is shipped as fp8 pre-scaled by 64


def _ln(nc, sb, x_ap, out_ap, eps_tile):
    """LayerNorm (gamma=1, beta=0): x_ap [128, D] f32 -> out_ap (bf16)."""
    stats = sb.tile([P, 2, 6], F32, tag="ln_stats")
    for g in range(2):
        nc.vector.bn_stats(out=stats[:, g, :], in_=x_ap[:, g * 512:(g + 1) * 512])
    mv = sb.tile([P, 2], F32, tag="ln_mv")
    nc.vector.bn_aggr(out=mv[:], in_=stats[:])
    rstd = sb.tile([P, 1], F32, tag="ln_rstd")
    nc.scalar.activation(out=rstd[:], in_=mv[:, 1:2], func=AF.Sqrt,
                         bias=eps_tile[:], scale=1.0)
    nc.vector.reciprocal(out=rstd[:], in_=rstd[:])
    nc.vector.tensor_scalar(out=out_ap, in0=x_ap, scalar1=mv[:, 0:1],
                            scalar2=rstd[:], op0=OP.subtract, op1=OP.mult)


def _softmax16(nc, sb, logits_ap, w_ap):
    """softmax over 16 router logits (PSUM f32 in) -> w_ap [128,16] f32."""
    mx = sb.tile([P, 1], F32, tag="rs_mx")
    nc.vector.tensor_reduce(out=mx[:], in_=logits_ap, axis=AX.X, op=OP.max)
    nmx = sb.tile([P, 1], F32, tag="rs_nmx")
    nc.vector.tensor_scalar_mul(out=nmx[:], in0=mx[:], scalar1=-1.0)
    ssum = sb.tile([P, 1], F32, tag="rs_sum")
    nc.scalar.activation(out=w_ap, in_=logits_ap, func=AF.Exp,
                         bias=nmx[:], scale=1.0, accum_out=ssum[:])
    nc.vector.reciprocal(out=ssum[:], in_=ssum[:])
    nc.vector.tensor_scalar_mul(out=w_ap, in0=w_ap, scalar1=ssum[:])


def _combine(nc, sb, p1_halves, w_ap, out_ap):
    """out[t,:] = sum_n w[t,n] * P1[t, n*128:(n+1)*128] (P1 in 2 PSUM halves)."""
    acc = sb.tile([P, R], F32, tag="cmb_acc")
    for n in range(NCMP):
        src = p1_halves[n // 8][:, (n % 8) * R:(n % 8 + 1) * R]
        if n == 0:
            nc.vector.tensor_scalar(out=acc[:], in0=src, scalar1=w_ap[:, 0:1],
                                    scalar2=None, op0=OP.mult)
        else:
            nc.vector.scalar_tensor_tensor(out=acc[:], in0=src,
                                           scalar=w_ap[:, n:n + 1], in1=acc[:],
                                           op0=OP.mult, op1=OP.add)
    nc.vector.tensor_copy(out=out_ap, in_=acc[:])


def build_program():
    nc = bacc.Bacc(None, num_devices=N_CORES)

    # single flat per-core weight-shard buffer: [neur | routers | W_O | WQKV]
    WSH_N = D // 8 * NCMP * R        # 262144
    WSH_R = D // 8 * 64              # 8192
    WSH_O = D // 8 * D               # 131072
    WSH_W = 48 * D                   # 49152
    WSH = WSH_N + WSH_R + WSH_O + WSH_W
    x_in = nc.dram_tensor("x_shard", [TOK, D], BF16, kind="ExternalInput")
    wts_in = nc.dram_tensor("wts_sh", [1, WSH], BF16, kind="ExternalInput")
    kKT_in = nc.dram_tensor("kKT", [R, NKS], F8, kind="ExternalInput")
    kV_in = nc.dram_tensor("kV", [NKS, D], F8, kind="ExternalInput")
    offs_in = nc.dram_tensor("offs", [1, 2], U32, kind="ExternalInput")
    out_t = nc.dram_tensor("out_shard", [TOK, D], BF16, kind="ExternalOutput")

    with tile.TileContext(nc) as tc:
        with (
            tc.tile_pool(name="persist", bufs=1) as pp,
            tc.tile_pool(name="weights", bufs=1) as wp,
            tc.tile_pool(name="work", bufs=2) as sb,
            tc.tile_pool(name="gath", bufs=3) as gp,
            tc.tile_pool(name="ps_big", bufs=2, space="PSUM") as psb,
            tc.tile_pool(name="ps_tp", bufs=2, space="PSUM") as pst,
            tc.tile_pool(name="ps_sm", bufs=2, space="PSUM") as psa,
            tc.tile_pool(name="dram", bufs=1, space="DRAM") as dram,
        ):
            # ---- parity offsets -> gpsimd registers for dynamic DMA slices ----
            r2048 = nc.gpsimd.alloc_register("off2048")
            nc.gpsimd.reg_load(r2048, offs_in[0:1, 0:1])
            off2048 = nc.gpsimd.snap(r2048, donate=True, min_val=0, max_val=2048)
            r512 = nc.gpsimd.alloc_register("off512")
            nc.gpsimd.reg_load(r512, offs_in[0:1, 1:2])
            off512 = nc.gpsimd.snap(r512, donate=True, min_val=0, max_val=512)

            group8 = [list(range(N_CORES))]

            # ---- broadcast replicated weights on-device (1/8 shard shipped
            # from host per core as one flat buffer; a single AllGather
            # reassembles every replicated weight in rank==row-block order) ----
            stgA = dram.tile([1, WSH], BF16)
            nc.gpsimd.dma_start(out=stgA[:], in_=wts_in[:])
            agA = dram.tile([N_CORES, WSH], BF16)
            nc.gpsimd.collective_compute("AllGather", OP.bypass,
                                         replica_groups=group8,
                                         ins=[stgA.opt()], outs=[agA.opt()])
            O_N, O_R, O_O, O_W = (0, WSH_N, WSH_N + WSH_R, WSH_N + WSH_R + WSH_O)

            # ---- resident weights ----
            neur = wp.tile([P, 8, NCMP * R], BF16)
            nc.sync.dma_start(
                out=neur[:],
                in_=agA[:, O_N:O_N + WSH_N].rearrange("c (p n) -> p c n", p=P))
            rtv = agA[:, O_R:O_R + WSH_R].rearrange("c (p n) -> p c n", p=P)
            rQKV = wp.tile([P, 8, 48], BF16)
            nc.sync.dma_start(out=rQKV[:], in_=rtv[:, :, 0:48])
            rM = wp.tile([P, 8, NCMP], BF16)
            nc.sync.dma_start(out=rM[:], in_=rtv[:, :, 48:64])
            wo = wp.tile([P, 8, D], BF16)
            nc.sync.dma_start(
                out=wo[:],
                in_=agA[:, O_O:O_O + WSH_O].rearrange("c (p n) -> p c n", p=P))
            wq = wp.tile([P, D], BF16)
            wk = wp.tile([P, D], BF16)
            wv = wp.tile([P, D], BF16)
            for c in range(N_CORES):
                for w_i, w_t in enumerate((wq, wk, wv)):
                    o0 = O_W + w_i * 16 * D
                    nc.sync.dma_start(
                        out=w_t[c * 16:(c + 1) * 16, :],
                        in_=agA[c:c + 1, o0:o0 + 16 * D].rearrange(
                            "r (p n) -> (r p) n", p=16))
            kkt = wp.tile([P, NKS], F8)
            nc.sync.dma_start(out=kkt[:], in_=kKT_in[:])
            eps_t = wp.tile([P, 1], F32)
            nc.vector.memset(eps_t[:], EPS)
            iota_t = wp.tile([P, KC], U16)
            nc.gpsimd.iota(out=iota_t[:], pattern=[[64, KC]], base=0,
                           channel_multiplier=0)

            # ---- generate ident (bf16 I) and tri (0 / -1e30 causal) ----
            colx = wp.tile([P, P], F32)
            nc.gpsimd.iota(out=colx[:], pattern=[[1, P]], base=0,
                           channel_multiplier=0,
                           allow_small_or_imprecise_dtypes=True)
            rowx = wp.tile([P, 1], F32)
            nc.gpsimd.iota(out=rowx[:], pattern=[[0, 1]], base=0,
                           channel_multiplier=1,
                           allow_small_or_imprecise_dtypes=True)
            ident = wp.tile([P, P], BF16)
            nc.vector.tensor_scalar(out=ident[:], in0=colx[:], scalar1=rowx[:],
                                    scalar2=None, op0=OP.is_equal)
            tri = wp.tile([P, P], F32)
            nc.vector.tensor_scalar(out=tri[:], in0=colx[:], scalar1=rowx[:],
                                    scalar2=NEG, op0=OP.is_gt, op1=OP.mult)

            # ---- persistent activations ----
            x_all = pp.tile([P, NT, D], F32)
            hT = pp.tile([P, 8, TOK], BF16, tag="hT")
            hQT = pp.tile([P, TOK], BF16, tag="hQT")
            hQT8 = pp.tile([P, TOK], F8, tag="hQT8")
            hKT = pp.tile([P, TOK], BF16, tag="hKT")
            hVT = pp.tile([P, TOK], BF16, tag="hVT")
            QT_sb = pp.tile([P, 8, TOK], BF16, tag="qt")
            KT_sb = pp.tile([P, 8, TOK], BF16, tag="kt")
            V_sb = pp.tile([P, NT, D], BF16, tag="vv")

            # packed score buffers (iota pre-written into the low u16 lanes)
            packed = [pp.tile([P, KC], U32, tag=f"pk{i}", name=f"pk{i}") for i in range(3)]
            for pk in packed:
                nc.vector.tensor_copy(out=pk.bitcast(U16)[:, 0::2], in_=iota_t[:])

            # =========== S1: LN1, shared projection, routed compress ===========
            for t in range(NT):
                ts = slice(t * P, (t + 1) * P)
                xb = sb.tile([P, D], BF16, tag="xb")
                nc.sync.dma_start(out=xb[:], in_=x_in[ts, :])
                nc.vector.tensor_copy(out=x_all[:, t, :], in_=xb[:])
                h = sb.tile([P, D], BF16, tag="h")
                _ln(nc, sb, x_all[:, t, :], h[:], eps_t)
                for ch in range(8):
                    tp = pst.tile([P, P], BF16, tag="tp")
                    nc.tensor.transpose(out=tp[:], in_=h[:, ch * P:(ch + 1) * P],
                                        identity=ident[:])
                    nc.scalar.activation(out=hT[:, ch, ts], in_=tp[:], func=AF.Copy)
                lg = psa.tile([P, 48], F32, tag="sm")
                for ch in range(8):
                    nc.tensor.matmul(out=lg[:], lhsT=hT[:, ch, ts], rhs=rQKV[:, ch, :],
                                     start=(ch == 0), stop=(ch == 7))
                wQKV = sb.tile([P, 48], F32, tag="wQKV")
                for rr in range(3):
                    _softmax16(nc, sb, lg[:, rr * 16:(rr + 1) * 16],
                               wQKV[:, rr * 16:(rr + 1) * 16])
                p1a = psb.tile([P, KC], F32, tag="big")
                p1b = psb.tile([P, KC], F32, tag="big")
                for half, pt in ((0, p1a), (1, p1b)):
                    for col in range(2):
                        c0 = half * KC + col * 512
                        for ch in range(8):
                            nc.tensor.matmul(out=pt[:, col * 512:(col + 1) * 512],
                                             lhsT=hT[:, ch, ts],
                                             rhs=neur[:, ch, c0:c0 + 512],
                                             start=(ch == 0), stop=(ch == 7))
                for rr, dst in ((0, hQT), (1, hKT), (2, hVT)):
                    hc = sb.tile([P, R], BF16, tag="hc")
                    _combine(nc, sb, (p1a, p1b), wQKV[:, rr * 16:(rr + 1) * 16], hc[:])
                    tp = pst.tile([P, P], BF16, tag="tp")
                    nc.tensor.transpose(out=tp[:], in_=hc[:], identity=ident[:])
                    nc.scalar.activation(out=dst[:, ts], in_=tp[:], func=AF.Copy)

            # =========== S2: Q^T / K^T (all 16 heads) and V ===========
            for ch in range(8):
                for w_, hsrc, dst in ((wq, hQT, QT_sb), (wk, hKT, KT_sb)):
                    pr = pst.tile([P, TOK], F32, tag="tp")
                    nc.tensor.matmul(out=pr[:], lhsT=w_[:, ch * P:(ch + 1) * P],
                                     rhs=hsrc[:], start=True, stop=True)
                    nc.scalar.activation(out=dst[:, ch, :], in_=pr[:], func=AF.Copy)
            for t in range(NT):
                pv = psb.tile([P, D], F32, tag="big")
                for col in range(2):
                    nc.tensor.matmul(out=pv[:, col * 512:(col + 1) * 512],
                                     lhsT=hVT[:, t * P:(t + 1) * P],
                                     rhs=wv[:, col * 512:(col + 1) * 512],
                                     start=True, stop=True)
                nc.scalar.activation(out=V_sb[:, t, :], in_=pv[:], func=AF.Copy)

            # =========== S3: pair AllGather of QT/KT/V ===========
            groups = [[0, 1], [2, 3], [4, 5], [6, 7]]
            xin = dram.tile([P, 12288], BF16)
            xout = dram.tile([2 * P, 12288], BF16)
            nc.gpsimd.dma_start(out=xin[:, 0:4096],
                                in_=QT_sb[:].rearrange("p c t -> p (c t)"))
            nc.gpsimd.dma_start(out=xin[:, 4096:8192],
                                in_=KT_sb[:].rearrange("p c t -> p (c t)"))
            nc.gpsimd.dma_start(out=xin[:, 8192:12288],
                                in_=V_sb[:].rearrange("p c t -> p (c t)"))
            nc.gpsimd.collective_compute("AllGather", OP.bypass,
                                         replica_groups=groups,
                                         ins=[xin.opt()], outs=[xout.opt()])
            # reuse the big persistent slots for the assembled full-seq tensors
            QT_f = pp.tile([P, 4, S], BF16, tag="qt")
            KT_f = pp.tile([P, 4, S], BF16, tag="kt")
            V_f = pp.tile([P, 8, 512], BF16, tag="vv")
            for src in range(2):
                rs = slice(src * P, (src + 1) * P)
                qsl = slice(src * TOK, (src + 1) * TOK)
                for i in range(4):
                    nc.gpsimd.dma_start(
                        out=QT_f[:, i, qsl],
                        in_=xout[rs, 0:4096][:, bass.ds(off2048 + i * TOK, TOK)])
                    nc.gpsimd.dma_start(
                        out=KT_f[:, i, qsl],
                        in_=xout[rs, 4096:8192][:, bass.ds(off2048 + i * TOK, TOK)])
                    nc.gpsimd.dma_start(
                        out=V_f[:, src * 4 + i, :],
                        in_=xout[rs, 8192:12288][:, bass.ds(off512 + i * D, 512)])

            # =========== S4: causal attention, 8 heads, full sequence ===========
            attnT = pp.tile([P, 4, S], BF16, tag="at")
            for hh in range(8):
                ch, poff = hh // 2, (hh % 2) * 64
                prow = slice(poff, poff + 64)
                for qg in range(8):
                    kr = (qg + 1) * P
                    sc = psb.tile([P, S], F32, tag="big")
                    for part in range((kr + 511) // 512):
                        k0, k1 = part * 512, min(kr, (part + 1) * 512)
                        nc.tensor.matmul(out=sc[:, k0:k1],
                                         lhsT=QT_f[prow, ch, qg * P:(qg + 1) * P],
                                         rhs=KT_f[prow, ch, k0:k1],
                                         start=True, stop=True)
                    mtmp = sb.tile([P, P], F32, tag="mtmp")
                    nc.vector.tensor_tensor(out=mtmp[:], in0=sc[:, qg * P:kr],
                                            in1=tri[:], op=OP.add)
                    Pb = sb.tile([P, S], BF16, tag="Pb")
                    s2 = sb.tile([P, 1], F32, tag="s2")
                    if qg > 0:
                        s1 = sb.tile([P, 1], F32, tag="s1")
                        nc.scalar.activation(out=Pb[:, 0:qg * P], in_=sc[:, 0:qg * P],
                                             func=AF.Exp, scale=0.125, accum_out=s1[:])
                    nc.scalar.activation(out=Pb[:, qg * P:kr], in_=mtmp[:],
                                         func=AF.Exp, scale=0.125, accum_out=s2[:])
                    den = sb.tile([P, 1], F32, tag="den")
                    if qg > 0:
                        nc.vector.tensor_tensor(out=den[:], in0=s1[:], in1=s2[:],
                                                op=OP.add)
                    else:
                        nc.vector.tensor_copy(out=den[:], in_=s2[:])
                    nc.vector.reciprocal(out=den[:], in_=den[:])
                    diag = sb.tile([P, P], BF16, tag="diag")
                    nc.vector.tensor_tensor(out=diag[:], in0=ident[:],
                                            in1=den[:].to_broadcast([P, P]),
                                            op=OP.mult)
                    at = psa.tile([64, P], F32, tag="sm")
                    for kb in range(qg + 1):
                        ptp = pst.tile([P, P], F32, tag="tp")
                        nc.tensor.matmul(out=ptp[:],
                                         lhsT=Pb[:, kb * P:(kb + 1) * P],
                                         rhs=diag[:], start=True, stop=True)
                        pts = sb.tile([P, P], BF16, tag="pts")
                        nc.scalar.activation(out=pts[:], in_=ptp[:], func=AF.Copy)
                        nc.tensor.matmul(out=at[:],
                                         lhsT=V_f[:, kb, hh * 64:(hh + 1) * 64],
                                         rhs=pts[:], start=(kb == 0), stop=(kb == qg))
                    nc.scalar.activation(out=attnT[prow, ch, qg * P:(qg + 1) * P],
                                         in_=at[:], func=AF.Copy)

            # =========== S5: exchange attn^T, W_O, residual ===========
            xin2 = dram.tile([P, 4 * S], BF16)
            xout2 = dram.tile([2 * P, 4 * S], BF16)
            nc.gpsimd.dma_start(out=xin2[:], in_=attnT[:].rearrange("p c q -> p (c q)"))
            nc.gpsimd.collective_compute("AllGather", OP.bypass,
                                         replica_groups=groups,
                                         ins=[xin2.opt()], outs=[xout2.opt()])
            aT = pp.tile([P, 8, TOK], BF16, tag="at")
            for src in range(2):
                rs = slice(src * P, (src + 1) * P)
                for i in range(4):
                    nc.gpsimd.dma_start(
                        out=aT[:, src * 4 + i, :],
                        in_=xout2[rs, :][:, bass.ds(off512 + i * S, TOK)])
            for t in range(NT):
                ts = slice(t * P, (t + 1) * P)
                po = psb.tile([P, D], F32, tag="big")
                for col in range(2):
                    for ch in range(8):
                        nc.tensor.matmul(out=po[:, col * 512:(col + 1) * 512],
                                         lhsT=aT[:, ch, ts],
                                         rhs=wo[:, ch, col * 512:(col + 1) * 512],
                                         start=(ch == 0), stop=(ch == 7))
                nc.vector.tensor_tensor(out=x_all[:, t, :], in0=po[:],
                                        in1=x_all[:, t, :], op=OP.add)

            # =========== S6: LN2 + compress M -> Qm^T (into hQT) ===========
            for t in range(NT):
                ts = slice(t * P, (t + 1) * P)
                h2 = sb.tile([P, D], BF16, tag="h")
                _ln(nc, sb, x_all[:, t, :], h2[:], eps_t)
                for ch in range(8):
                    tp = pst.tile([P, P], BF16, tag="tp")
                    nc.tensor.transpose(out=tp[:], in_=h2[:, ch * P:(ch + 1) * P],
                                        identity=ident[:])
                    nc.scalar.activation(out=hT[:, ch, ts], in_=tp[:], func=AF.Copy)
                lgm = psa.tile([P, NCMP], F32, tag="sm")
                for ch in range(8):
                    nc.tensor.matmul(out=lgm[:], lhsT=hT[:, ch, ts], rhs=rM[:, ch, :],
                                     start=(ch == 0), stop=(ch == 7))
                wM = sb.tile([P, NCMP], F32, tag="wM")
                _softmax16(nc, sb, lgm[:], wM[:])
                p1a = psb.tile([P, KC], F32, tag="big")
                p1b = psb.tile([P, KC], F32, tag="big")
                for half, pt in ((0, p1a), (1, p1b)):
                    for col in range(2):
                        c0 = half * KC + col * 512
                        for ch in range(8):
                            nc.tensor.matmul(out=pt[:, col * 512:(col + 1) * 512],
                                             lhsT=hT[:, ch, ts],
                                             rhs=neur[:, ch, c0:c0 + 512],
                                             start=(ch == 0), stop=(ch == 7))
                qm = sb.tile([P, R], BF16, tag="hc")
                _combine(nc, sb, (p1a, p1b), wM[:], qm[:])
                tp = pst.tile([P, P], BF16, tag="tp")
                nc.tensor.transpose(out=tp[:], in_=qm[:], identity=ident[:])
                nc.scalar.activation(out=hQT8[:, ts], in_=tp[:], func=AF.Copy)

            # =========== S7a: AllGather Qm^T across all 8 cores ===========
            xin3 = dram.tile([P, TOK], F8)
            xout3 = dram.tile([N_CORES * P, TOK], F8)
            nc.gpsimd.dma_start(out=xin3[:], in_=hQT8[:])
            nc.gpsimd.collective_compute("AllGather", OP.bypass,
                                         replica_groups=group8,
                                         ins=[xin3.opt()], outs=[xout3.opt()])
            QmT_f = pp.tile([P, N_CORES, TOK], F8, tag="qt")
            for c in range(N_CORES):
                nc.gpsimd.dma_start(out=QmT_f[:, c, :],
                                    in_=xout3[c * P:(c + 1) * P, :])

            # ===== S7b: scores vs local shard + local top-8, all 32 tiles =====
            cands = pp.tile([P, NTT, NKC * 8], U32, tag="cands")
            top8a = pp.tile([P, NTT, 8], U32, tag="top8a")
            for q in range(NTT):
                lq = QmT_f[:, q // 4, (q % 4) * P:(q % 4 + 1) * P]
                for ch in range(NKC):
                    ks = psb.tile([P, KC], F32, tag="big")
                    for col in range(2):
                        c0 = ch * KC + col * 512
                        nc.tensor.matmul(out=ks[:, col * 512:(col + 1) * 512],
                                         lhsT=lq, rhs=kkt[:, c0:c0 + 512],
                                         start=True, stop=True)
                    pk = packed[(q * NKC + ch) % 3]
                    nc.scalar.activation(out=pk.bitcast(U16)[:, 1::2].bitcast(BF16),
                                         in_=ks[:], func=AF.Copy)
                    c8 = cands[:, q, ch * 8:(ch + 1) * 8]
                    nc.vector.max(out=c8.bitcast(F32), in_=pk.bitcast(F32)[:])
                t8 = top8a[:, q, :]
                nc.vector.max(out=t8.bitcast(F32), in_=cands.bitcast(F32)[:, q, :])

            # =========== S7c: AllGather packed top-8 candidates ===========
            xin4 = dram.tile([P, NTT * 8], U32)
            xout4 = dram.tile([N_CORES * P, NTT * 8], U32)
            nc.gpsimd.dma_start(out=xin4[:],
                                in_=top8a[:].rearrange("p t s -> p (t s)"))
            nc.gpsimd.collective_compute("AllGather", OP.bypass,
                                         replica_groups=group8,
                                         ins=[xin4.opt()], outs=[xout4.opt()])
            cand_all = pp.tile([P, NTT, N_CORES * 8], U32, tag="hT")
            for c in range(N_CORES):
                nc.sync.dma_start(
                    out=cand_all[:, :, c * 8:(c + 1) * 8],
                    in_=xout4[c * P:(c + 1) * P, :].rearrange("p (t s) -> p t s", s=8))

            # ==== S7d: per-token softmax stats, my weights, decode my idx ====
            m8_all = pp.tile([P, NTT, 8], F32, tag="m8a")
            for q in range(NTT):
                nc.vector.max(out=m8_all[:, q, :], in_=cand_all.bitcast(F32)[:, q, :])
            # all-candidate scores, exp, threshold mask, Z
            # (scores are tiny, |s| < 1, so the usual max-subtraction before
            # exp is unnecessary; softmax is shift-invariant)
            s_all = pp.tile([P, NTT, N_CORES * 8], F32, tag="kt")
            nc.vector.tensor_scalar(out=s_all[:].bitcast(U32), in0=cand_all[:],
                                    scalar1=0xFFFF0000, scalar2=None,
                                    op0=OP.bitwise_and)
            ex_all = pp.tile([P, NTT, N_CORES * 8], F32, tag="vv")
            nc.scalar.activation(out=ex_all[:], in_=s_all[:], func=AF.Exp,
                                 scale=SCALE_K)
            mask_all = pp.tile([P, NTT, N_CORES * 8], F32, tag="mska")
            nc.vector.tensor_tensor(out=mask_all[:], in0=cand_all.bitcast(F32)[:],
                                    in1=m8_all[:, :, 7:8].to_broadcast(
                                        [P, NTT, N_CORES * 8]),
                                    op=OP.is_ge)
            nc.vector.tensor_tensor(out=ex_all[:], in0=ex_all[:], in1=mask_all[:],
                                    op=OP.mult)
            zz = pp.tile([P, NTT, 1], F32, tag="zz")
            nc.vector.tensor_reduce(out=zz[:], in_=ex_all[:], axis=AX.X, op=OP.add)
            nc.vector.reciprocal(out=zz[:], in_=zz[:])
            # fold the 1/64 fp8 kV pre-scale into the softmax normalizer
            nc.vector.tensor_scalar_mul(out=zz[:], in0=zz[:], scalar1=1.0 / 64.0)
            # my candidates: scores, exp, mask, weights
            s8a = pp.tile([P, NTT, 8], F32, tag="s8a")
            nc.vector.tensor_scalar(out=s8a[:].bitcast(U32), in0=top8a[:],
                                    scalar1=0xFFFF0000, scalar2=None,
                                    op0=OP.bitwise_and)
            w8_all = pp.tile([P, NTT, 8], F32, tag="w8a")
            nc.scalar.activation(out=w8_all[:], in_=s8a[:], func=AF.Exp,
                                 scale=SCALE_K)
            msk8 = pp.tile([P, NTT, 8], F32, tag="msk8")
            nc.vector.tensor_tensor(out=msk8[:], in0=top8a.bitcast(F32)[:],
                                    in1=m8_all[:, :, 7:8].to_broadcast([P, NTT, 8]),
                                    op=OP.is_ge)
            nc.vector.tensor_tensor(out=w8_all[:], in0=w8_all[:], in1=msk8[:],
                                    op=OP.mult)
            nc.vector.tensor_tensor(out=w8_all[:], in0=w8_all[:],
                                    in1=zz[:].to_broadcast([P, NTT, 8]),
                                    op=OP.mult)
            # decode my local knowledge-row indices
            pos_all = pp.tile([P, NTT, 8], U32, tag="posa")
            for q in range(NTT):
                nc.vector.max_index(out=pos_all[:, q, :],
                                    in_max=top8a.bitcast(F32)[:, q, :],
                                    in_values=cands.bitcast(F32)[:, q, :])
            idx_all = pp.tile([P, NTT, 8], U32, tag="idxa")
            nc.vector.tensor_scalar(out=idx_all[:], in0=pos_all[:],
                                    scalar1=3, scalar2=10,
                                    op0=OP.logical_shift_right,
                                    op1=OP.logical_shift_left)
            loc_all = pp.tile([P, NTT, 8], U32, tag="loca")
            nc.vector.tensor_scalar(out=loc_all[:], in0=top8a[:],
                                    scalar1=6, scalar2=0x3FF,
                                    op0=OP.logical_shift_right,
                                    op1=OP.bitwise_and)
            nc.vector.tensor_tensor(out=idx_all[:], in0=idx_all[:], in1=loc_all[:],
                                    op=OP.bitwise_or)

            # ==== S7e: gather my V rows, weighted partials, ReduceScatter ====
            rsin = dram.tile([NTT * P, D], BF16)
            rsout = dram.tile([NT * P, D], BF16)
            for q in range(NTT):
                acc = sb.tile([P, D], F32, tag="acc")
                for j in range(KK):
                    vg = gp.tile([P, D], F8, tag="vg8")
                    nc.gpsimd.indirect_dma_start(
                        out=vg[:], out_offset=None, in_=kV_in[:],
                        in_offset=bass.IndirectOffsetOnAxis(
                            ap=idx_all[:, q, j:j + 1], axis=0))
                    if j == 0:
                        nc.vector.tensor_scalar(out=acc[:], in0=vg[:],
                                                scalar1=w8_all[:, q, 0:1],
                                                scalar2=None, op0=OP.mult)
                    else:
                        nc.vector.scalar_tensor_tensor(out=acc[:], in0=vg[:],
                                                       scalar=w8_all[:, q, j:j + 1],
                                                       in1=acc[:], op0=OP.mult,
                                                       op1=OP.add)
                accb = sb.tile([P, D], BF16, tag="accb")
                nc.scalar.activation(out=accb[:], in_=acc[:], func=AF.Copy)
                nc.sync.dma_start(out=rsin[q * P:(q + 1) * P, :], in_=accb[:])
            nc.gpsimd.collective_compute("ReduceScatter", OP.add,
                                         replica_groups=group8,
                                         ins=[rsin.opt()], outs=[rsout.opt()])
            for t in range(NT):
                ts = slice(t * P, (t + 1) * P)
                mem = gp.tile([P, D], BF16, tag="vg")
                nc.sync.dma_start(out=mem[:], in_=rsout[t * P:(t + 1) * P, :])
                outsb = sb.tile([P, D], BF16, tag="outsb")
                nc.vector.tensor_tensor(out=outsb[:], in0=mem[:],
                                        in1=x_all[:, t, :], op=OP.add)
                nc.sync.dma_start(out=out_t[ts, :], in_=outsb[:])

    nc.finalize()
    return nc


@functools.lru_cache(maxsize=1)
def _get_program():
    return build_program()


def _prep_core_inputs(inputs):
    bf = ml_dtypes.bfloat16
    x = np.asarray(inputs["x"], np.float32).astype(bf)
    neurons = np.asarray(inputs["compress_neurons"], np.float32)
    neur_flat = np.ascontiguousarray(
        neurons.transpose(1, 0, 2).reshape(D, NCMP * R)).astype(bf)
    rt_full = np.concatenate([np.asarray(inputs["router_Q"], np.float32),
                              np.asarray(inputs["router_K"], np.float32),
                              np.asarray(inputs["router_V"], np.float32),
                              np.asarray(inputs["router_M"], np.float32)],
                             axis=1).astype(bf)
    wq = np.asarray(inputs["W_Q"], np.float32).astype(bf)
    wk = np.asarray(inputs["W_K"], np.float32).astype(bf)
    wv = np.asarray(inputs["W_V"], np.float32).astype(bf)
    wo = np.asarray(inputs["W_O"], np.float32).astype(bf)
    kKT8_full = np.ascontiguousarray(
        np.asarray(inputs["knowledge_K"], np.float32).T * 64.0).astype(
        ml_dtypes.float8_e4m3)
    kV8_full = (np.asarray(inputs["knowledge_V"], np.float32) * 64.0).astype(
        ml_dtypes.float8_e4m3)
    in_maps = []
    for c in range(N_CORES):
        b, hf = c // 2, c % 2
        rs = slice(c * (D // 8), (c + 1) * (D // 8))
        ws = slice(c * 16, (c + 1) * 16)
        m = dict(
            x_shard=np.ascontiguousarray(x[b, hf * TOK:(hf + 1) * TOK, :]),
            offs=np.array([[hf * 2048, hf * 512]], np.uint32),
            wts_sh=np.concatenate(
                [neur_flat[rs, :].ravel(), rt_full[rs, :].ravel(),
                 wo[rs, :].ravel(), wq[ws, :].ravel(), wk[ws, :].ravel(),
                 wv[ws, :].ravel()])[None, :],
            kKT=np.ascontiguousarray(kKT8_full[:, c * NKS:(c + 1) * NKS]),
            kV=np.ascontiguousarray(kV8_full[c * NKS:(c + 1) * NKS, :]),
        )
        in_maps.append(m)
    return in_maps


def kernel(**inputs) -> np.ndarray:
    nc = _get_program()
    in_maps = _prep_core_inputs(inputs)
    res = run_bass_kernel_spmd(nc, in_maps, list(range(N_CORES)))
    out = np.empty((B, S, D), np.float32)
    for c in range(N_CORES):
        b, hf = c // 2, c % 2
        out[b, hf * TOK:(hf + 1) * TOK, :] = np.asarray(
            res.results[c]["out_shard"], dtype=np.float32)
    return out
